# revision 1
# baseline (speedup 1.0000x reference)
"""Distributed Trainium2 Bass kernel for AdaptedAttention (LLaMA-Adapter style).

Sharding: pure data-parallel over the B*S = 8192 token axis (1024 tokens per
core across 8 NeuronCores).  The adapter attention only attends to the L=64
adapter slots, so there is no cross-token dependency; each core produces its
own slice of the output.  Adapter K/V projections are computed per 4-head
shard on each core and AllGathered (tiny: 2*64KB bf16).  Wq/Wo are replicated
and streamed from HBM.

Precision: Q-path matmuls in bf16 (fp32 PSUM); the O-projection runs in
fp8e4m3 with DoubleRow perf mode (2 weights/cell, K=256 per instruction,
half-rate streaming) — its error bypasses the softmax and is diluted by the
exact-f32 base_output add, keeping total rel err ~1e-3.

Host-side prep: RoPE cos/sin tables from position_ids, 1/sqrt(D) folded into
Wq, adaption_gate folded into Wv, fp8 scale factors folded into Wo and
compensated in the final add; all tensors pre-tiled/transposed so the device
never transposes and every DMA is a large contiguous burst.

Device pipeline per core (single fused graph):
  - iter 0..31: per head h: qT = WqT_h^T @ xT (PSUM), qa = q*cos, qb = q*sin'
    (rotate-half eliminated: scores contract over head dim, so
    scores = KT^T qa + KTrowswap^T qb), qa/qb parked in DRAM.
  - iters 1-2 interleave the adapter K/V shard matmuls + AllGather; the
    attention stages run LAG=16 heads behind, so the collective has ~250us
    of slack and core start skew never stalls the (in-order) TensorE stream.
  - stages (lagged, 1 head apart each): scores+exp -> ones-matmul sum +
    reciprocal -> partition_broadcast + probs -> aoT_h = V_h^T probs (fp8).
  - phase C: outT = (WoT^T aoT) * 1/S + baseT via fp8 DoubleRow matmuls;
    host transposes outT back.
"""

import numpy as np
import ml_dtypes

B, S, HID = 4, 2048, 4096
H, D, L = 32, 128, 64
NCORES = 8
T = B * S
TC = T // NCORES          # tokens per core (1024)
KC = HID // 128           # 32 contraction chunks over hidden dim
HS = H // NCORES          # adapter heads computed locally per core (4)
ROPE_THETA = 10000.0

S_A = 64.0                # fp8 scale on aoT
S_W = 1024.0              # fp8 scale on WoT
OSCALE = 1.0 / (S_A * S_W)
S_X = 16.0                # fp8 scale on xT
S_Q = 8192.0              # fp8 scale on WqT (1/sqrt(D) already folded)
S_P = 16.0                # fp8 scale on qa/qb (rope products)
S_K = 16.0                # fp8 scale on adapter KT
QSCALE = S_P / (S_X * S_Q)   # folded into the cos/sin tables on host
ESCALE = 1.0 / (S_P * S_K)   # descale via the exp activation's scale arg

_cache = {}


def _build(tc_tokens=TC):
    """Builds the SPMD Bass graph (identical on all 8 cores)."""
    import concourse.tile as tile
    from concourse import bacc, mybir
    from contextlib import ExitStack

    bf16 = mybir.dt.bfloat16
    fp8 = mybir.dt.float8e4
    f32 = mybir.dt.float32
    MUL = mybir.AluOpType.mult
    ADD = mybir.AluOpType.add
    EXP = mybir.ActivationFunctionType.Exp

    MB = tc_tokens // 512       # 512-token m-chunks (2)
    assert tc_tokens % 512 == 0

    nc = bacc.Bacc(
        "TRN2",
        target_bir_lowering=False,
        debug=False,
        enable_asserts=False,
        num_devices=NCORES,
    )

    # Host-pretiled layouts (every DMA a large contiguous burst):
    #   xT   [128, KC*tc]    : [p, k*tc + m] = x.T[128k+p, m]
    #   wqT  [H*128, KC*128] : [128h+p, 128k+c] = Wq.T[128k+p, 128h+c]
    #   woT  [KC*128, KC*128]: fp8 DoubleRow: [128n+p, 256k2+128i+c]
    #                          = Wo.T[256k2+128i+p, 128n+c] * S_W
    #   wkTs/wvTs [128, KC*HS*D], pT [128, KC*L] : [p, (k n)] tiling
    xT = nc.dram_tensor("xT", [128, KC * tc_tokens], fp8, kind="ExternalInput").ap()
    baseT = nc.dram_tensor("baseT", [HID, tc_tokens], f32, kind="ExternalInput").ap()
    wqT = nc.dram_tensor("wqT", [H * 128, KC * 128], fp8, kind="ExternalInput").ap()
    woT = nc.dram_tensor("woT", [KC * 128, KC * 128], fp8, kind="ExternalInput").ap()
    wkTs = nc.dram_tensor("wkTs", [128, KC * HS * D], bf16, kind="ExternalInput").ap()
    wvTs = nc.dram_tensor("wvTs", [128, KC * HS * D], bf16, kind="ExternalInput").ap()
    pT = nc.dram_tensor("pT", [128, KC * L], bf16, kind="ExternalInput").ap()
    cosT = nc.dram_tensor("cosT", [D, tc_tokens], bf16, kind="ExternalInput").ap()
    sinT = nc.dram_tensor("sinT", [D, tc_tokens], bf16, kind="ExternalInput").ap()
    outT = nc.dram_tensor("outT", [HID, tc_tokens], f32, kind="ExternalOutput").ap()

    with tile.TileContext(nc) as tc, ExitStack() as ctx:
        const_pool = ctx.enter_context(tc.tile_pool(name="const", bufs=1))
        persist = ctx.enter_context(tc.tile_pool(name="persist", bufs=1))

        # ---- persistent SBUF residents ----
        xT_sb = persist.tile([128, KC * tc_tokens], fp8)
        aoT_sb = persist.tile([128, KC * tc_tokens], fp8)
        cos_sb = persist.tile([128, tc_tokens], bf16)
        sin_sb = persist.tile([128, tc_tokens], bf16)
        KT_sb = persist.tile([128, H * L], bf16)             # head h at cols 64h
        KTs_sb = persist.tile([128, H * L], bf16)            # row-swapped KT
        KTp_sb = persist.tile([128, H * 2 * L], fp8)         # [KT|KTs] interleaved
        V_sb = persist.tile([64, H * D], bf16)               # head h at cols 128h
        ones64 = const_pool.tile([64, 1], bf16)
        nc.gpsimd.memset(ones64[:], 1.0)

        # ============ Phase B (with phase A interleaved at iters 1-2) ====
        LAG = 16
        with tc.tile_pool(name="wq", bufs=2) as wqp, \
             tc.tile_pool(name="rope", bufs=2) as rp, \
             tc.tile_pool(name="qrd", bufs=8) as qrd, \
             tc.tile_pool(name="attn", bufs=4) as asb, \
             tc.tile_pool(name="pa_sb", bufs=1) as pa, \
             tc.tile_pool(name="pa_w", bufs=3) as paw, \
             tc.tile_pool(name="qdram", bufs=1, space="DRAM") as qdp, \
             tc.tile_pool(name="cc_dram", bufs=1, space="DRAM") as dram, \
             tc.tile_pool(name="qps", bufs=2, space="PSUM") as qpsp, \
             tc.tile_pool(name="scps", bufs=2, space="PSUM") as scp, \
             tc.tile_pool(name="sups", bufs=2, space="PSUM") as sup, \
             tc.tile_pool(name="aops", bufs=2, space="PSUM") as aop:
            qaD = qdp.tile([H * 128, tc_tokens], fp8)
            qbD = qdp.tile([H * 128, tc_tokens], fp8)
            pT_sb = pa.tile([128, KC * L], bf16)
            ktl_sb = pa.tile([128, HS * L], bf16)
            vl_sb = pa.tile([64, HS * D], bf16)
            NW = HS * D
            CCF = 128 * HS * L
            cc_in = dram.tile([2, CCF], bf16)
            cc_out = dram.tile([NCORES, 2, CCF], bf16, addr_space="Shared")

            nc.sync.dma_start(cos_sb[:], cosT[:])
            nc.sync.dma_start(sin_sb[:], sinT[:])
            nc.sync.dma_start(xT_sb[:], xT[:])
            nc.sync.dma_start(pT_sb[:], pT[:])
            # DoubleRow rhs layout: pair (i) blocks contiguous per m-chunk so
            # the moving operand is one linear 1024-value run
            xT_r = xT_sb.rearrange("p (k q i m) -> p k q i m",
                                   k=KC // 2, q=MB, i=2)

            def kv_pass(jh):
                # adapter K shard (2 head-dim chunks) + V shard (jh==0);
                # PSUM borrowed from the sc/ao tags (idle until iter LAG)
                ktps = [scp.tile([128, L], f32, tag="sc", name=f"ktp{jh}_{t}")
                        for t in range(2)]
                vps = None
                if jh == 0:
                    vps = aop.tile([64, NW], f32, tag="ao", name="vps")
                for k in range(KC):
                    wk_h = paw.tile([128, 256], bf16, tag="wk")
                    nc.sync.dma_start(
                        wk_h[:],
                        wkTs[:, NW * k + 256 * jh:NW * k + 256 * (jh + 1)])
                    st, sp = (k == 0), (k == KC - 1)
                    for t in range(2):
                        nc.tensor.matmul(
                            ktps[t][:], wk_h[:, 128 * t:128 * (t + 1)],
                            pT_sb[:, L * k:L * (k + 1)], start=st, stop=sp)
                    if jh == 0:
                        wv_h = paw.tile([128, NW], bf16, tag="wv")
                        nc.sync.dma_start(wv_h[:], wvTs[:, NW * k:NW * (k + 1)])
                        nc.tensor.matmul(
                            vps[:], pT_sb[:, L * k:L * (k + 1)], wv_h[:],
                            start=st, stop=sp)
                for t in range(2):
                    j = 2 * jh + t
                    nc.scalar.copy(ktl_sb[:, L * j:L * (j + 1)], ktps[t][:])
                if jh == 0:
                    nc.scalar.copy(vl_sb[:], vps[:])

            def collective():
                nc.sync.dma_start(cc_in[0].rearrange("(p f) -> p f", p=128),
                                  ktl_sb[:])
                nc.sync.dma_start(cc_in[1].rearrange("(p f) -> p f", p=64),
                                  vl_sb[:])
                nc.gpsimd.collective_compute(
                    "AllGather",
                    mybir.AluOpType.bypass,
                    replica_groups=[list(range(NCORES))],
                    ins=[cc_in[:].opt()],
                    outs=[cc_out[:].opt()],
                )
                for c in range(NCORES):
                    cs = slice(c * HS * L, (c + 1) * HS * L)
                    ktc = cc_out[c, 0].rearrange("(p f) -> p f", p=128)
                    nc.sync.dma_start(KT_sb[:, cs], ktc)
                    # rotate-half as a row-swap in the gather-back DMA
                    nc.sync.dma_start(KTs_sb[0:64, cs], ktc[64:128, :])
                    nc.sync.dma_start(KTs_sb[64:128, cs], ktc[0:64, :])
                    nc.sync.dma_start(
                        V_sb[:, c * HS * D:(c + 1) * HS * D],
                        cc_out[c, 1].rearrange("(p f) -> p f", p=64))
                # fp8 DoubleRow stationary for the scores matmul: per head
                # [KT*S_K | KTs*S_K] as the two K-groups
                ktp_v = KTp_sb.rearrange("p (h i l) -> p h i l", h=H, i=2)
                nc.vector.tensor_scalar_mul(
                    ktp_v[:, :, 0, :],
                    KT_sb.rearrange("p (h l) -> p h l", h=H), S_K)
                nc.vector.tensor_scalar_mul(
                    ktp_v[:, :, 1, :],
                    KTs_sb.rearrange("p (h l) -> p h l", h=H), S_K)

            qab_st, esb_st, rec_st, probs_st = {}, {}, {}, {}

            def stage0(j):      # prefetch qa/qb back from DRAM (interleaved)
                pair = []
                for m in range(MB):
                    ms = slice(512 * m, 512 * (m + 1))
                    qab = qrd.tile([128, 1024], fp8, tag="qab",
                                   name=f"qab{j}_{m}")
                    nc.sync.dma_start(qab[:, 0:512],
                                      qaD[128 * j:128 * (j + 1), ms])
                    nc.sync.dma_start(qab[:, 512:1024],
                                      qbD[128 * j:128 * (j + 1), ms])
                    pair.append(qab)
                qab_st[j] = pair

            def stage1(j):      # scores (one fp8 DoubleRow mm: both RoPE
                                # arms as the two K-groups) + descaled exp
                qab = qab_st.pop(j)
                kt_h = KTp_sb.rearrange("p (h i l) -> p h i l", h=H, i=2)[:, j]
                for m in range(MB):
                    sc = scp.tile([64, 512], f32, tag="sc", name=f"sc{j}_{m}")
                    nc.tensor.matmul(
                        sc[:], kt_h,
                        qab[m].rearrange("p (i m) -> p i m", i=2),
                        start=True, stop=True,
                        perf_mode=mybir.MatmulPerfMode.DoubleRow,
                    )
                    esb = asb.tile([64, 512], bf16, tag="esb",
                                   name=f"esb{j}_{m}", bufs=10)
                    nc.scalar.activation(esb[:], sc[:], EXP, scale=ESCALE)
                    esb_st[(j, m)] = esb

            def stage2(j):      # partition-sum + reciprocal
                for m in range(MB):
                    sums = sup.tile([1, 512], f32, tag="sums", name=f"su{j}_{m}")
                    nc.tensor.matmul(sums[:], ones64[:], esb_st[(j, m)][:],
                                     start=True, stop=True)
                    rec = asb.tile([1, 512], bf16, tag="rec", name=f"re{j}_{m}", bufs=8)
                    with nc.allow_low_precision(reason="bf16 softmax weights"):
                        nc.vector.reciprocal(rec[:], sums[:])
                    rec_st[(j, m)] = rec

            def stage3(j):      # broadcast + probs
                for m in range(MB):
                    bc = asb.tile([64, 512], bf16, tag="bc", name=f"bc{j}_{m}",
                                  bufs=4)
                    nc.gpsimd.partition_broadcast(bc[:], rec_st.pop((j, m))[:])
                    probs = asb.tile([64, 512], bf16, tag="probs",
                                     name=f"pr{j}_{m}", bufs=8)
                    nc.vector.tensor_tensor(probs[:], esb_st.pop((j, m))[:],
                                            bc[:], MUL)
                    probs_st[(j, m)] = probs

            def stage4(j):      # adapter output, scaled fp8 copy to aoT
                # aoT stored in DoubleRow rhs layout: col =
                # (j//2)*2048 + m*1024 + (j%2)*512  (pair blocks contiguous)
                for m in range(MB):
                    ao = aop.tile([128, 512], f32, tag="ao", name=f"ao{j}_{m}")
                    nc.tensor.matmul(
                        ao[:], V_sb[:, D * j:D * (j + 1)],
                        probs_st.pop((j, m))[:], start=True, stop=True,
                    )
                    base_col = (j // 2) * 2048 + m * 1024 + (j % 2) * 512
                    nc.scalar.mul(
                        aoT_sb[:, base_col:base_col + 512], ao[:], S_A,
                    )

            for it in range(H):
                if it == 1:
                    kv_pass(0)
                elif it == 2:
                    kv_pass(1)
                    collective()
                if 0 <= it - (LAG - 1) < H:
                    stage0(it - (LAG - 1))
                if 0 <= it - LAG < H:
                    stage1(it - LAG)
                if 0 <= it - LAG - 1 < H:
                    stage2(it - LAG - 1)
                if 0 <= it - LAG - 2 < H:
                    stage3(it - LAG - 2)
                if 0 <= it - LAG - 3 < H:
                    stage4(it - LAG - 3)
                h = it
                wq_sb = wqp.tile([128, KC * 128], fp8, tag="wq")
                nc.sync.dma_start(wq_sb[:], wqT[128 * h:128 * (h + 1), :])
                wq_r = wq_sb.rearrange("p (k i c) -> p k i c", k=KC // 2, i=2)
                # fp8 DoubleRow, k-outer / m-inner: K=256 per instruction at
                # half-rate streaming; scale folded into cos/sin tables
                qps = [qpsp.tile([128, 512], f32, tag="qp", name=f"qp{h}_{m}")
                       for m in range(MB)]
                for k2 in range(KC // 2):
                    for m in range(MB):
                        nc.tensor.matmul(
                            qps[m][:],
                            wq_r[:, k2],
                            xT_r[:, k2, m],
                            start=(k2 == 0), stop=(k2 == KC // 2 - 1),
                            perf_mode=mybir.MatmulPerfMode.DoubleRow,
                        )
                for m in range(MB):
                    ms = slice(512 * m, 512 * (m + 1))
                    # RoPE, rotate-half-free: scores contract over head dim,
                    # so scores = KT^T (q*cos) + KTswap^T (q*sin'); products
                    # stored fp8 (scale S_P folded into the host tables)
                    qa = rp.tile([128, 512], fp8, tag="qa", name=f"qa{h}_{m}")
                    nc.vector.tensor_tensor(qa[:], qps[m][:], cos_sb[:, ms], MUL)
                    qb = rp.tile([128, 512], fp8, tag="qb", name=f"qb{h}_{m}")
                    nc.vector.tensor_tensor(qb[:], qps[m][:], sin_sb[:, ms], MUL)
                    nc.sync.dma_start(qaD[128 * h:128 * (h + 1), ms], qa[:])
                    nc.sync.dma_start(qbD[128 * h:128 * (h + 1), ms], qb[:])

            # Epilogue: drain the lagged stages two heads per round to
            # halve the tail's dependency-chain overhead (no Qproj left
            # to hide behind).
            stages = [stage0, stage1, stage2, stage3, stage4]
            ptrs = [H - LAG + 1, H - LAG, H - LAG - 1, H - LAG - 2,
                    H - LAG - 3]
            while any(p < H for p in ptrs):
                for s in range(5):
                    for _ in range(2):
                        if ptrs[s] < H and (s == 0 or ptrs[s] < ptrs[s - 1]):
                            stages[s](ptrs[s])
                            ptrs[s] += 1

        # ================= Phase C: fp8 DoubleRow O-proj + base add ======
        with tc.tile_pool(name="wo", bufs=2) as wop, \
             tc.tile_pool(name="fin", bufs=3) as fin, \
             tc.tile_pool(name="ops", bufs=4, space="PSUM") as opp:
            aoT_r = aoT_sb.rearrange("p (k q i m) -> p k q i m",
                                     k=KC // 2, q=MB, i=2)
            K2 = KC // 2
            for ni in range(KC):
                wo_sb = wop.tile([128, KC * 128], fp8, tag="wo")
                nc.sync.dma_start(wo_sb[:], woT[128 * ni:128 * (ni + 1), :])
                wo_r = wo_sb.rearrange("p (k i c) -> p k i c", k=K2, i=2)
                ops = [opp.tile([128, 512], f32, tag="op", name=f"op{ni}_{m}")
                       for m in range(MB)]
                for k2 in range(K2):
                    for m in range(MB):
                        nc.tensor.matmul(
                            ops[m][:],
                            wo_r[:, k2],
                            aoT_r[:, k2, m],
                            start=(k2 == 0), stop=(k2 == K2 - 1),
                            perf_mode=mybir.MatmulPerfMode.DoubleRow,
                        )
                for m in range(MB):
                    bt = fin.tile([128, 512], f32, tag="bt")
                    nc.sync.dma_start(
                        bt[:],
                        baseT[128 * ni:128 * (ni + 1), 512 * m:512 * (m + 1)],
                    )
                    osb = fin.tile([128, 512], f32, tag="osb")
                    nc.vector.scalar_tensor_tensor(
                        osb[:], ops[m][:], OSCALE, bt[:], MUL, ADD)
                    nc.sync.dma_start(
                        outT[128 * ni:128 * (ni + 1), 512 * m:512 * (m + 1)],
                        osb[:],
                    )

    nc.compile()
    return nc


def _host_prep(hidden_states, base_output, Wq, Wk, Wv, Wo, adaption_prompt,
               adaption_gate, position_ids, tc_tokens=TC, ncores=NCORES):
    bf16 = ml_dtypes.bfloat16
    fp8 = ml_dtypes.float8_e4m3
    f32 = np.float32

    x = np.ascontiguousarray(np.asarray(hidden_states, f32).reshape(T, HID))
    base = np.asarray(base_output, f32).reshape(T, HID)
    pos = np.asarray(position_ids).reshape(T).astype(np.int64)

    inv = 1.0 / (ROPE_THETA ** (np.arange(0, D, 2, dtype=f32) / D))
    freqs = pos[:, None].astype(f32) * inv[None, :]          # [T, 64]
    emb = np.concatenate([freqs, freqs], axis=1)             # [T, 128]
    # QSCALE compensates the fp8 scaling of the Q projection inputs
    cos = (np.cos(emb) * QSCALE).astype(f32)
    sin = (np.sin(emb) * QSCALE).astype(f32)
    # sin arm pairs with the row-swapped KT: +sin (p<64), -sin (p>=64)
    sin_signed = sin.copy()
    sin_signed[:, D // 2:] *= -1.0

    gate = f32(np.asarray(adaption_gate).reshape(-1)[0])
    scale = f32(1.0 / np.sqrt(D))

    def tile_kp(A):
        # A [HID, N] -> [128, KC*N] with [p, k*N + n] = A[128k+p, n]
        n = A.shape[1]
        return np.ascontiguousarray(
            A.reshape(KC, 128, n).transpose(1, 0, 2).reshape(128, KC * n))

    def tile_blocks(A):
        # A [HID, HID] -> [H*128, KC*128] with [128b+p, 128k+c] = A[128k+p, 128b+c]
        return np.ascontiguousarray(
            A.reshape(KC, 128, KC, 128).transpose(2, 1, 0, 3)
             .reshape(KC * 128, KC * 128))

    def tile_doublerow(A):
        # A [HID, HID] -> [KC*128, KC*128] with
        # [128n+p, 256k2+128i+c] = A[256k2+128i+p, 128n+c]
        return np.ascontiguousarray(
            A.reshape(KC // 2, 2, 128, KC, 128).transpose(3, 2, 0, 1, 4)
             .reshape(KC * 128, KC * 128))

    def tile_dr_rhs(A):
        # A [HID, N] -> [128, KC*N], cols (k2, mc, i, m):
        # [p, k2*2N + mc*1024 + i*512 + m] = A[256k2+128i+p, 512mc+m]
        n = A.shape[1]
        return np.ascontiguousarray(
            A.reshape(KC // 2, 2, 128, n // 512, 512)
             .transpose(2, 0, 3, 1, 4).reshape(128, KC * n))

    WqT = tile_doublerow(np.asarray(Wq, f32).T * (scale * f32(S_Q))).astype(fp8)
    WoT = tile_doublerow(np.asarray(Wo, f32).T * f32(S_W)).astype(fp8)
    WkT = np.asarray(Wk, f32).T.astype(bf16)
    WvT = (np.asarray(Wv, f32).T * gate).astype(bf16)
    pTn = tile_kp(np.asarray(adaption_prompt, f32).reshape(L, HID).T
                  .astype(bf16))

    in_maps = []
    for c in range(ncores):
        lo = c * tc_tokens
        hi = lo + tc_tokens
        hd = slice(c * HS * D, (c + 1) * HS * D)
        in_maps.append({
            "xT": tile_dr_rhs((x[lo:hi].T * f32(S_X)).astype(fp8)),
            "baseT": np.ascontiguousarray(base[lo:hi].T),
            "wqT": WqT,
            "woT": WoT,
            "wkTs": tile_kp(WkT[:, hd]),
            "wvTs": tile_kp(WvT[:, hd]),
            "pT": pTn,
            "cosT": np.ascontiguousarray(cos[lo:hi].T).astype(bf16),
            "sinT": np.ascontiguousarray(sin_signed[lo:hi].T).astype(bf16),
        })
    return in_maps


def kernel(hidden_states, base_output, Wq, Wk, Wv, Wo, adaption_prompt,
           adaption_gate, position_ids):
    from concourse import bass_utils

    if "nc" not in _cache:
        _cache["nc"] = _build()
    nc = _cache["nc"]

    in_maps = _host_prep(hidden_states, base_output, Wq, Wk, Wv, Wo,
                         adaption_prompt, adaption_gate, position_ids)

    res = bass_utils.run_bass_kernel_spmd(nc, in_maps, core_ids=list(range(NCORES)))

    out = np.empty((T, HID), np.float32)
    for c in range(NCORES):
        out[c * TC:(c + 1) * TC] = res.results[c]["outT"].T
    return out.reshape(B, S, HID)



# revision 10
# speedup vs baseline: 1.7145x; 1.7145x over previous
"""Distributed Trainium2 Bass kernel for AdaptedAttention (LLaMA-Adapter style).

Sharding: pure data-parallel over the B*S = 8192 token axis (1024 tokens per
core across 8 NeuronCores).  The adapter attention only attends to the L=64
adapter slots, so there is no cross-token dependency; each core produces its
own slice of the output.

Weight folding (host, numpy): the adapter K and the product V@Wo depend only
on weight-type inputs (adaption_prompt, Wk, Wv, Wo, adaption_gate), so they
are folded on the host like the other weight transforms (gate into Wv, scale
into Wq, RoPE tables).  Folding V@Wo per head gives a [H*L=2048, HID] output
projection, halving the O-proj FLOPs (the contraction drops 4096 -> 2048) and
removing the per-head probs@V matmuls entirely.

Device pipeline per core (all big matmuls fp8e4m3 DoubleRow, K=256/instr):
  - per head pair a (heads 2a, 2a+1): qT = WqT^T @ xT (PSUM), RoPE arms
    qa=q*cos, qb=q*sin' as fp8 (rotate-half eliminated: scores contract over
    the head dim, so scores = KT^T qa + KTrowswap^T qb, one DoubleRow mm with
    [KT|KTs] as the two K-groups); both heads of a pair land in one
    [128,512] PSUM tile -> one Exp activation.
  - softmax sums batched: an indicator-matrix matmul accumulates per-head
    column sums of exp into a shared [16,512] PSUM tile (value 1/64 folds the
    fp8 probs scale); one reciprocal per 16-head group replaces 64 tiny ones.
  - reciprocal rows are partition-broadcast with a second indicator matmul
    (PE, ~0.2us) and multiplied into probs (fp8, scale 64).
  - O-proj: outT = (VWo^T probsT) * OSCALE + baseT via fp8 DoubleRow matmuls
    over the (head,slot) contraction.
All stages are software-pipelined (scores lag 1 pair, sums lag 2, probs of
group 0 interleave with the Qproj of group 1) so TensorE never waits on
Scalar/Vector.
"""

import numpy as np
import ml_dtypes

B, S, HID = 4, 2048, 4096
H, D, L = 32, 128, 64
NCORES = 8
T = B * S
TC = T // NCORES          # tokens per core (1024)
KC = HID // 128           # 32 contraction chunks over hidden dim
HL = H * L                # folded O-proj contraction (2048)
MB = TC // 512            # 512-token m-chunks (2)
ROPE_THETA = 10000.0

S_X = 16.0                # fp8 scale on xT
S_Q = 8192.0              # fp8 scale on WqT (1/sqrt(D) already folded)
S_P = 16.0                # fp8 scale on qa/qb (rope products)
S_K = 16.0                # fp8 scale on adapter KT
S_PR = 64.0               # fp8 scale on probs (folded via 1/64 indicator)
S_VW = 1024.0             # fp8 scale on VWo
QSCALE = S_P / (S_X * S_Q)   # folded into the cos/sin tables on host
ESCALE = 1.0 / (S_P * S_K)   # descale via the exp activation's scale arg
OSCALE = 1.0 / (S_PR * S_VW)

_cache = {}


def _build(tc_tokens=TC):
    """Builds the SPMD Bass graph (identical on all 8 cores)."""
    import concourse.tile as tile
    from concourse import bacc, mybir
    from contextlib import ExitStack

    bf16 = mybir.dt.bfloat16
    fp8 = mybir.dt.float8e4
    f32 = mybir.dt.float32
    MUL = mybir.AluOpType.mult
    ADD = mybir.AluOpType.add
    EXP = mybir.ActivationFunctionType.Exp
    DR = mybir.MatmulPerfMode.DoubleRow

    assert tc_tokens == TC and MB == 2

    nc = bacc.Bacc(
        "TRN2",
        target_bir_lowering=False,
        debug=False,
        enable_asserts=False,
    )

    # Host-pretiled layouts (every DMA a large contiguous burst):
    #   xT    [128, KC*tc]   : DoubleRow rhs tiling (see tile_dr_rhs)
    #   wqT   [H*128, KC*128]: [128h+p, 256k2+128i+c] = (Wq.T*s)[256k2+128i+p, 128h+c]
    #   ktpT  [128, H*2*L]   : [p, 128h+64i+l] = KT_h[p,l] (i=0) / KT_h[(p+64)%128,l] (i=1)
    #   vwoT  [KC*128, HL]   : [128n+p, 256b+128i+c] = (VWo*s)[256b+128i+p, 128n+c]
    xT = nc.dram_tensor("xT", [128, KC * TC], fp8, kind="ExternalInput").ap()
    cosT = nc.dram_tensor("cosT", [D, TC], bf16, kind="ExternalInput").ap()
    sinT = nc.dram_tensor("sinT", [D, TC], bf16, kind="ExternalInput").ap()
    wqT = nc.dram_tensor("wqT", [H * 128, KC * 128], fp8, kind="ExternalInput").ap()
    ktpT = nc.dram_tensor("ktpT", [128, H * 2 * L], fp8, kind="ExternalInput").ap()
    vwoT = nc.dram_tensor("vwoT", [KC * 128, HL], fp8, kind="ExternalInput").ap()
    baseT = nc.dram_tensor("baseT", [HID, TC], bf16, kind="ExternalInput").ap()
    u2T = nc.dram_tensor("u2T", [128, 32], bf16, kind="ExternalInput").ap()
    z2T = nc.dram_tensor("z2T", [16, 16 * 64], bf16, kind="ExternalInput").ap()
    outT = nc.dram_tensor("outT", [HID, TC], f32, kind="ExternalOutput").ap()

    with tile.TileContext(nc) as tc, ExitStack() as ctx:
        const_pool = ctx.enter_context(tc.tile_pool(name="const", bufs=1))
        persist = ctx.enter_context(tc.tile_pool(name="persist", bufs=1))

        # ---- persistent SBUF residents ----
        xT_sb = persist.tile([128, KC * TC], fp8)
        cos_sb = persist.tile([128, TC], bf16)
        sin_sb = persist.tile([128, TC], bf16)
        ktp_sb = persist.tile([128, H * 2 * L], fp8)
        probsT_sb = persist.tile([128, 16 * 1024], fp8)   # [p, 1024a+512m+t]

        # sums indicator (host const): col 15 = upper-half ones, col 16 =
        # lower-half ones; value 1/64 folds the fp8 probs scale.
        U2 = const_pool.tile([128, 32], bf16)
        nc.sync.dma_start(U2[:], u2T[:])
        # broadcast indicator (host const): Z2[k, c] = 1 iff k == c//64
        Z2 = const_pool.tile([16, 16 * 64], bf16)
        nc.sync.dma_start(Z2[:], z2T[:])

        nc.sync.dma_start(cos_sb[:], cosT[:])
        nc.sync.dma_start(sin_sb[:], sinT[:])
        nc.sync.dma_start(ktp_sb[:], ktpT[:])
        # xT split per k2-chunk so the first Qproj matmuls start early
        for k2 in range(KC // 2):
            cs = slice(2048 * k2, 2048 * (k2 + 1))
            nc.sync.dma_start(xT_sb[:, cs], xT[:, cs])

        xT_r = xT_sb.rearrange("p (k q i m) -> p k q i m", k=KC // 2, q=MB, i=2)
        ktp_r = ktp_sb.rearrange("p (h i l) -> p h i l", h=H, i=2)

        # ============ Phase B: Qproj + RoPE + scores + softmax ============
        with tc.tile_pool(name="wq", bufs=2) as wqp, \
             tc.tile_pool(name="rope", bufs=10) as rp, \
             tc.tile_pool(name="esb", bufs=34) as esbp, \
             tc.tile_pool(name="recs", bufs=4) as recp, \
             tc.tile_pool(name="qps", bufs=2, space="PSUM") as qpsp, \
             tc.tile_pool(name="scps", bufs=2, space="PSUM") as scp, \
             tc.tile_pool(name="sups", bufs=2, space="PSUM") as sup, \
             tc.tile_pool(name="bcps", bufs=2, space="PSUM") as bcp:

            qab_st, esb_st, sums_st, rec_st = {}, {}, {}, {}

            def qproj(a):
                for j in (2 * a, 2 * a + 1):
                    wq_sb = wqp.tile([128, KC * 128], fp8, tag="wq")
                    nc.sync.dma_start(wq_sb[:], wqT[128 * j:128 * (j + 1), :])
                    wq_r = wq_sb.rearrange("p (k i c) -> p k i c",
                                           k=KC // 2, i=2)
                    for m in range(MB):
                        qp = qpsp.tile([128, 512], f32, tag="qp",
                                       name=f"qp{j}_{m}")
                        for k2 in range(KC // 2):
                            nc.tensor.matmul(
                                qp[:], wq_r[:, k2], xT_r[:, k2, m],
                                start=(k2 == 0), stop=(k2 == KC // 2 - 1),
                                perf_mode=DR,
                            )
                        ms = slice(512 * m, 512 * (m + 1))
                        qab = rp.tile([128, 1024], fp8, tag="qab",
                                      name=f"qab{j}_{m}")
                        nc.vector.tensor_tensor(qab[:, 0:512], qp[:],
                                                cos_sb[:, ms], MUL)
                        nc.vector.tensor_tensor(qab[:, 512:1024], qp[:],
                                                sin_sb[:, ms], MUL)
                        qab_st[(j, m)] = qab

            def scores(a):
                # DoubleRow can't col-tile (XBUS budget), so the two RoPE
                # arms accumulate as two plain fp8 matmuls per head; the two
                # heads of a pair land in col-groups 0-63 / 64-127.
                for m in range(MB):
                    psc = scp.tile([128, 512], f32, tag="sc", name=f"sc{a}_{m}")
                    for i, j in enumerate((2 * a, 2 * a + 1)):
                        qab = qab_st.pop((j, m))
                        for arm in range(2):
                            nc.tensor.matmul(
                                psc[64 * i:64 * (i + 1), :],
                                ktp_r[:, j, arm, :],
                                qab[:, 512 * arm:512 * (arm + 1)],
                                start=(arm == 0), stop=(arm == 1),
                            )
                    esb = esbp.tile([128, 512], bf16, tag="esb",
                                    name=f"esb{a}_{m}")
                    nc.scalar.activation(esb[:], psc[:], EXP, scale=ESCALE)
                    esb_st[(a, m)] = esb

            def sums(a):
                g, al = a // 8, a % 8
                for m in range(MB):
                    if al == 0:
                        sums_st[(g, m)] = sup.tile([16, 512], f32, tag="sums",
                                                   name=f"su{g}_{m}")
                    nc.tensor.matmul(
                        sums_st[(g, m)][:], U2[:, 15 - 2 * al:31 - 2 * al],
                        esb_st[(a, m)][:], start=(al == 0), stop=(al == 7),
                    )

            def rec(g):
                for m in range(MB):
                    rc = recp.tile([16, 512], bf16, tag="rec",
                                   name=f"rec{g}_{m}")
                    with nc.allow_low_precision(reason="bf16 softmax weights"):
                        nc.vector.reciprocal(rc[:], sums_st[(g, m)][:])
                    rec_st[(g, m)] = rc

            def bc_probs(a):
                g, al = a // 8, a % 8
                for m in range(MB):
                    pb = bcp.tile([128, 512], f32, tag="bcp", name=f"bc{a}_{m}")
                    nc.tensor.matmul(pb[:], Z2[:, 128 * al:128 * (al + 1)],
                                     rec_st[(g, m)][:], start=True, stop=True)
                    ps = slice(1024 * a + 512 * m, 1024 * a + 512 * (m + 1))
                    nc.vector.tensor_tensor(probsT_sb[:, ps],
                                            esb_st.pop((a, m))[:], pb[:], MUL)

            for a in range(16):
                qproj(a)
                if a >= 1:
                    scores(a - 1)
                if a == 10:
                    rec(0)      # before sums(8) so group-0 PSUM slots free
                if a >= 2:
                    sums(a - 2)
                if a >= 12:
                    bc_probs(2 * (a - 12))
                    bc_probs(2 * (a - 12) + 1)
            scores(15)
            sums(14)
            sums(15)
            rec(1)
            for a in range(8, 16):
                bc_probs(a)

        # ================= Phase C: fp8 DoubleRow O-proj + base add ======
        with tc.tile_pool(name="wo", bufs=2) as wop, \
             tc.tile_pool(name="fin", bufs=3) as fin, \
             tc.tile_pool(name="ops", bufs=4, space="PSUM") as opp:
            # probsT col = 1024a + 512m + t, a = 2b+i
            pr_r = probsT_sb.rearrange("p (b i m t) -> p b i m t",
                                       b=8, i=2, m=MB)
            for ni in range(KC):
                wo_sb = wop.tile([128, HL], fp8, tag="wo")
                nc.sync.dma_start(wo_sb[:], vwoT[128 * ni:128 * (ni + 1), :])
                wo_r = wo_sb.rearrange("p (b i c) -> p b i c", b=8, i=2)
                for m in range(MB):
                    op = opp.tile([128, 512], f32, tag="op",
                                  name=f"op{ni}_{m}")
                    for b in range(8):
                        nc.tensor.matmul(
                            op[:], wo_r[:, b], pr_r[:, b, :, m, :],
                            start=(b == 0), stop=(b == 7), perf_mode=DR,
                        )
                    ts = slice(512 * m, 512 * (m + 1))
                    bt = fin.tile([128, 512], bf16, tag="bt")
                    nc.sync.dma_start(
                        bt[:], baseT[128 * ni:128 * (ni + 1), ts])
                    osb = fin.tile([128, 512], f32, tag="osb")
                    nc.vector.scalar_tensor_tensor(
                        osb[:], op[:], OSCALE, bt[:], MUL, ADD)
                    nc.sync.dma_start(
                        outT[128 * ni:128 * (ni + 1), ts], osb[:])

    nc.compile()
    return nc


def _host_prep(hidden_states, base_output, Wq, Wk, Wv, Wo, adaption_prompt,
               adaption_gate, position_ids, tc_tokens=TC, ncores=NCORES):
    bf16 = ml_dtypes.bfloat16
    fp8 = ml_dtypes.float8_e4m3
    f32 = np.float32

    def to_fp8(a):
        return np.clip(a, -240.0, 240.0).astype(fp8)

    x = np.ascontiguousarray(np.asarray(hidden_states, f32).reshape(T, HID))
    base = np.asarray(base_output, f32).reshape(T, HID)
    pos = np.asarray(position_ids).reshape(T).astype(np.int64)

    inv = 1.0 / (ROPE_THETA ** (np.arange(0, D, 2, dtype=f32) / D))
    freqs = pos[:, None].astype(f32) * inv[None, :]          # [T, 64]
    emb = np.concatenate([freqs, freqs], axis=1)             # [T, 128]
    # QSCALE compensates the fp8 scaling of the Q projection inputs
    cos = (np.cos(emb) * QSCALE).astype(f32)
    sin = (np.sin(emb) * QSCALE).astype(f32)
    # sin arm pairs with the row-swapped KT: +sin (d<64), -sin (d>=64)
    sin_signed = sin.copy()
    sin_signed[:, D // 2:] *= -1.0

    gate = f32(np.asarray(adaption_gate).reshape(-1)[0])
    scale = f32(1.0 / np.sqrt(D))
    prompt = np.asarray(adaption_prompt, f32).reshape(L, HID)

    def tile_doublerow(A):
        # A [K, N] -> [N, K] tiled: [128n+p, 256b+128i+c] = A[256b+128i+p, 128n+c]
        K, N = A.shape
        return np.ascontiguousarray(
            A.reshape(K // 256, 2, 128, N // 128, 128).transpose(3, 2, 0, 1, 4)
             .reshape(N, K))

    def tile_dr_rhs(A):
        # A [HID, n] -> [128, KC*n], cols (k2, mc, i, m):
        # [p, k2*2n + mc*1024 + i*512 + m] = A[256k2+128i+p, 512mc+m]
        n = A.shape[1]
        return np.ascontiguousarray(
            A.reshape(KC // 2, 2, 128, n // 512, 512)
             .transpose(2, 0, 3, 1, 4).reshape(128, KC * n))

    WqT = tile_doublerow(np.asarray(Wq, f32).T * (scale * f32(S_Q)))
    WqT = to_fp8(WqT)

    # adapter K (host fold): K = prompt @ Wk.T, per head [KT | KT row-swapped]
    Kmat = prompt @ np.asarray(Wk, f32).T                    # [L, HID]
    KT = Kmat.reshape(L, H, D).transpose(2, 1, 0)            # [D, H, L]
    ktp = np.empty((128, H, 2, L), f32)
    ktp[:, :, 0, :] = KT * S_K
    ktp[:, :, 1, :] = np.roll(KT, -D // 2, axis=0) * S_K
    ktpT = to_fp8(ktp.reshape(128, H * 2 * L))

    # folded V@Wo (host): VWo[h] = (prompt @ Wv.T * gate)[:, h] @ Wo.T[h]
    V = (prompt @ np.asarray(Wv, f32).T) * gate              # [L, HID]
    V5 = V.reshape(L, H, D).transpose(1, 0, 2)               # [H, L, D]
    WoT5 = np.asarray(Wo, f32).T.reshape(H, D, HID)          # [H, D, HID]
    M2 = (V5 @ WoT5).reshape(HL, HID)                        # [(h,l), HID]
    vwoT = to_fp8(tile_doublerow(M2 * f32(S_VW)))

    u2 = np.zeros((128, 32), f32)
    u2[0:64, 15] = 1.0 / S_PR
    u2[64:128, 16] = 1.0 / S_PR
    z2 = np.zeros((16, 16 * 64), f32)
    for k in range(16):
        z2[k, 64 * k:64 * (k + 1)] = 1.0
    u2 = u2.astype(bf16)
    z2 = z2.astype(bf16)

    in_maps = []
    for c in range(ncores):
        lo = c * tc_tokens
        hi = lo + tc_tokens
        in_maps.append({
            "xT": tile_dr_rhs(to_fp8(x[lo:hi].T * f32(S_X))),
            "cosT": np.ascontiguousarray(cos[lo:hi].T).astype(bf16),
            "sinT": np.ascontiguousarray(sin_signed[lo:hi].T).astype(bf16),
            "wqT": WqT,
            "ktpT": ktpT,
            "vwoT": vwoT,
            "baseT": np.ascontiguousarray(base[lo:hi].T).astype(bf16),
            "u2T": u2,
            "z2T": z2,
        })
    return in_maps


def kernel(hidden_states, base_output, Wq, Wk, Wv, Wo, adaption_prompt,
           adaption_gate, position_ids):
    from concourse import bass_utils

    if "nc" not in _cache:
        _cache["nc"] = _build()
    nc = _cache["nc"]

    in_maps = _host_prep(hidden_states, base_output, Wq, Wk, Wv, Wo,
                         adaption_prompt, adaption_gate, position_ids)

    res = bass_utils.run_bass_kernel_spmd(nc, in_maps, core_ids=list(range(NCORES)))

    out = np.empty((T, HID), np.float32)
    for c in range(NCORES):
        out[c * TC:(c + 1) * TC] = res.results[c]["outT"].T
    return out.reshape(B, S, HID)


# revision 15
# speedup vs baseline: 1.8925x; 1.1039x over previous
"""Distributed Trainium2 Bass kernel for AdaptedAttention (LLaMA-Adapter style).

Sharding: pure data-parallel over the B*S = 8192 token axis (1024 tokens per
core across 8 NeuronCores).  The adapter attention only attends to the L=64
adapter slots, so there is no cross-token dependency; each core produces its
own slice of the output.

Weight folding (host, numpy): the adapter K and the product V@Wo depend only
on weight-type inputs (adaption_prompt, Wk, Wv, Wo, adaption_gate), so they
are folded on the host like the other weight transforms (gate into Wv, scale
into Wq, RoPE tables).  Folding V@Wo per head gives a [H*L=2048, HID] output
projection, halving the O-proj FLOPs (the contraction drops 4096 -> 2048) and
removing the per-head probs@V matmuls entirely.

Device pipeline per core (all big matmuls fp8e4m3 DoubleRow, K=256/instr):
  - per head pair a (heads 2a, 2a+1): qT = WqT^T @ xT (PSUM), RoPE arms
    qa=q*cos, qb=q*sin' as fp8 (rotate-half eliminated: scores contract over
    the head dim, so scores = KT^T qa + KTrowswap^T qb, one DoubleRow mm with
    [KT|KTs] as the two K-groups); both heads of a pair land in one
    [128,512] PSUM tile -> one Exp activation.
  - softmax sums batched: an indicator-matrix matmul accumulates per-head
    column sums of exp into a shared [16,512] PSUM tile (value 1/64 folds the
    fp8 probs scale); one reciprocal per 16-head group replaces 64 tiny ones.
  - reciprocal rows are partition-broadcast with a second indicator matmul
    (PE, ~0.2us) and multiplied into probs (fp8, scale 64).
  - O-proj: outT = (VWo^T probsT) * OSCALE + baseT via fp8 DoubleRow matmuls
    over the (head,slot) contraction.
All stages are software-pipelined (scores lag 1 pair, sums lag 2, probs of
group 0 interleave with the Qproj of group 1) so TensorE never waits on
Scalar/Vector.
"""

import numpy as np
import ml_dtypes

B, S, HID = 4, 2048, 4096
H, D, L = 32, 128, 64
NCORES = 8
T = B * S
TC = T // NCORES          # tokens per core (1024)
KC = HID // 128           # 32 contraction chunks over hidden dim
HL = H * L                # folded O-proj contraction (2048)
MB = TC // 512            # 512-token m-chunks (2)
ROPE_THETA = 10000.0

S_X = 16.0                # fp8 scale on xT
S_Q = 8192.0              # fp8 scale on WqT (1/sqrt(D) already folded)
S_P = 16.0                # fp8 scale on qa/qb (rope products)
S_K = 16.0                # fp8 scale on adapter KT
S_PR = 64.0               # fp8 scale on probs (folded via 1/64 indicator)
S_VW = 1024.0             # fp8 scale on VWo
QSCALE = S_P / (S_X * S_Q)   # folded into the cos/sin tables on host
ESCALE = 1.0 / (S_P * S_K)   # descale via the exp activation's scale arg
OSCALE = 1.0 / (S_PR * S_VW)

_cache = {}


def _build(tc_tokens=TC):
    """Builds the SPMD Bass graph (identical on all 8 cores)."""
    import concourse.tile as tile
    from concourse import bacc, mybir
    from contextlib import ExitStack

    bf16 = mybir.dt.bfloat16
    fp8 = mybir.dt.float8e4
    f32 = mybir.dt.float32
    MUL = mybir.AluOpType.mult
    ADD = mybir.AluOpType.add
    EXP = mybir.ActivationFunctionType.Exp
    DR = mybir.MatmulPerfMode.DoubleRow

    assert tc_tokens == TC and MB == 2

    nc = bacc.Bacc(
        "TRN2",
        target_bir_lowering=False,
        debug=False,
        enable_asserts=False,
    )

    # Host-pretiled layouts (every DMA a large contiguous burst):
    #   xT    [128, KC*tc]   : DoubleRow rhs tiling (see tile_dr_rhs)
    #   wqT   [H*128, KC*128]: [128h+p, 256k2+128i+c] = (Wq.T*s)[256k2+128i+p, 128h+c]
    #   ktpT  [128, H*2*L]   : [p, 128h+64i+l] = KT_h[p,l] (i=0) / KT_h[(p+64)%128,l] (i=1)
    #   vwoT  [KC*128, HL]   : [128n+p, 256b+128i+c] = (VWo*s)[256b+128i+p, 128n+c]
    xT = nc.dram_tensor("xT", [128, KC * TC], fp8, kind="ExternalInput").ap()
    cosT = nc.dram_tensor("cosT", [D, TC], bf16, kind="ExternalInput").ap()
    sinT = nc.dram_tensor("sinT", [D, TC], bf16, kind="ExternalInput").ap()
    wqT = nc.dram_tensor("wqT", [H * 128, KC * 128], fp8, kind="ExternalInput").ap()
    ktpT = nc.dram_tensor("ktpT", [128, H * 2 * L], fp8, kind="ExternalInput").ap()
    vwoT = nc.dram_tensor("vwoT", [KC * 128, HL], fp8, kind="ExternalInput").ap()
    baseT = nc.dram_tensor("baseT", [HID, TC], bf16, kind="ExternalInput").ap()
    u2T = nc.dram_tensor("u2T", [128, 32], bf16, kind="ExternalInput").ap()
    z2T = nc.dram_tensor("z2T", [16, 16 * 64], bf16, kind="ExternalInput").ap()
    outT = nc.dram_tensor("outT", [HID, TC], f32, kind="ExternalOutput").ap()

    with tile.TileContext(nc) as tc, ExitStack() as ctx:
        const_pool = ctx.enter_context(tc.tile_pool(name="const", bufs=1))
        persist = ctx.enter_context(tc.tile_pool(name="persist", bufs=1))

        # ---- persistent SBUF residents ----
        xT_sb = persist.tile([128, KC * TC], fp8)
        cos_sb = persist.tile([128, TC], bf16)
        sin_sb = persist.tile([128, TC], bf16)
        ktp_sb = persist.tile([128, H * 2 * L], fp8)
        probsT_sb = persist.tile([128, 16 * 1024], fp8)   # [p, 1024a+512m+t]
        vwo_sb = persist.tile([128, KC * HL], fp8)        # [p, 2048ni+col]
        U2 = const_pool.tile([128, 32], bf16)
        Z2 = const_pool.tile([16, 16 * 64], bf16)

        # First Qproj matmul needs only xT chunk 0 + wq head 0: issue those
        # DMAs first (descriptor issue serializes at ~0.6us each).
        nc.sync.dma_start(xT_sb[:, 0:2048], xT[:, 0:2048])

        def late_loads():
            nc.sync.dma_start(cos_sb[:], cosT[:])
            nc.sync.dma_start(sin_sb[:], sinT[:])
            nc.sync.dma_start(ktp_sb[:], ktpT[:])
            nc.sync.dma_start(U2[:], u2T[:])
            nc.sync.dma_start(Z2[:], z2T[:])
            for k2 in range(1, KC // 2):
                cs = slice(2048 * k2, 2048 * (k2 + 1))
                nc.sync.dma_start(xT_sb[:, cs], xT[:, cs])

        xT_r = xT_sb.rearrange("p (k q i m) -> p k q i m", k=KC // 2, q=MB, i=2)
        ktp_r = ktp_sb.rearrange("p (h i l) -> p h i l", h=H, i=2)

        # ============ Phase B: Qproj + RoPE + scores + softmax ============
        with tc.tile_pool(name="wq", bufs=2) as wqp, \
             tc.tile_pool(name="rope", bufs=10) as rp, \
             tc.tile_pool(name="esb", bufs=34) as esbp, \
             tc.tile_pool(name="recs", bufs=4) as recp, \
             tc.tile_pool(name="qps", bufs=2, space="PSUM") as qpsp, \
             tc.tile_pool(name="scps", bufs=2, space="PSUM") as scp, \
             tc.tile_pool(name="sups", bufs=2, space="PSUM") as sup, \
             tc.tile_pool(name="bcps", bufs=2, space="PSUM") as bcp:

            qab_st, esb_st, sums_st, rec_st, wq_st = {}, {}, {}, {}, {}

            def wq_load(j):
                wq_sb = wqp.tile([128, KC * 128], fp8, tag="wq")
                nc.sync.dma_start(wq_sb[:], wqT[128 * j:128 * (j + 1), :])
                wq_st[j] = wq_sb

            def qproj(a):
                for j in (2 * a, 2 * a + 1):
                    wq_sb = wq_st.pop(j)
                    wq_r = wq_sb.rearrange("p (k i c) -> p k i c",
                                           k=KC // 2, i=2)
                    for m in range(MB):
                        qp = qpsp.tile([128, 512], f32, tag="qp",
                                       name=f"qp{j}_{m}")
                        for k2 in range(KC // 2):
                            nc.tensor.matmul(
                                qp[:], wq_r[:, k2], xT_r[:, k2, m],
                                start=(k2 == 0), stop=(k2 == KC // 2 - 1),
                                perf_mode=DR,
                            )
                        ms = slice(512 * m, 512 * (m + 1))
                        qab = rp.tile([128, 1024], fp8, tag="qab",
                                      name=f"qab{j}_{m}")
                        nc.vector.tensor_tensor(qab[:, 0:512], qp[:],
                                                cos_sb[:, ms], MUL)
                        nc.vector.tensor_tensor(qab[:, 512:1024], qp[:],
                                                sin_sb[:, ms], MUL)
                        qab_st[(j, m)] = qab

            def scores(a):
                # DoubleRow can't col-tile (XBUS budget), so the two RoPE
                # arms accumulate as two plain fp8 matmuls per head; the two
                # heads of a pair land in col-groups 0-63 / 64-127.
                for m in range(MB):
                    psc = scp.tile([128, 512], f32, tag="sc", name=f"sc{a}_{m}")
                    for i, j in enumerate((2 * a, 2 * a + 1)):
                        qab = qab_st.pop((j, m))
                        for arm in range(2):
                            nc.tensor.matmul(
                                psc[64 * i:64 * (i + 1), :],
                                ktp_r[:, j, arm, :],
                                qab[:, 512 * arm:512 * (arm + 1)],
                                start=(arm == 0), stop=(arm == 1),
                            )
                    esb = esbp.tile([128, 512], bf16, tag="esb",
                                    name=f"esb{a}_{m}")
                    nc.scalar.activation(esb[:], psc[:], EXP, scale=ESCALE)
                    esb_st[(a, m)] = esb

            def sums(a):
                g, al = a // 8, a % 8
                for m in range(MB):
                    if al == 0:
                        sums_st[(g, m)] = sup.tile([16, 512], f32, tag="sums",
                                                   name=f"su{g}_{m}")
                    nc.tensor.matmul(
                        sums_st[(g, m)][:], U2[:, 15 - 2 * al:31 - 2 * al],
                        esb_st[(a, m)][:], start=(al == 0), stop=(al == 7),
                    )

            def rec(g):
                for m in range(MB):
                    rc = recp.tile([16, 512], bf16, tag="rec",
                                   name=f"rec{g}_{m}")
                    with nc.allow_low_precision(reason="bf16 softmax weights"):
                        nc.vector.reciprocal(rc[:], sums_st[(g, m)][:])
                    rec_st[(g, m)] = rc

            def bc_probs(a):
                g, al = a // 8, a % 8
                for m in range(MB):
                    pb = bcp.tile([128, 512], f32, tag="bcp", name=f"bc{a}_{m}")
                    nc.tensor.matmul(pb[:], Z2[:, 128 * al:128 * (al + 1)],
                                     rec_st[(g, m)][:], start=True, stop=True)
                    ps = slice(1024 * a + 512 * m, 1024 * a + 512 * (m + 1))
                    nc.vector.tensor_tensor(probsT_sb[:, ps],
                                            esb_st.pop((a, m))[:], pb[:], MUL)

            wq_load(0)
            wq_load(1)
            late_loads()
            for a in range(16):
                qproj(a)
                if a < 15:      # prefetch next pair's weights
                    wq_load(2 * a + 2)
                    wq_load(2 * a + 3)
                # stream the resident O-proj weights in during phase B
                for ni in (2 * a, 2 * a + 1):
                    nc.sync.dma_start(
                        vwo_sb[:, HL * ni:HL * (ni + 1)],
                        vwoT[128 * ni:128 * (ni + 1), :])
                if a >= 1:
                    scores(a - 1)
                if a == 10:
                    rec(0)      # before sums(8) so group-0 PSUM slots free
                if a >= 2:
                    sums(a - 2)
                if a >= 12:
                    bc_probs(2 * (a - 12))
                    bc_probs(2 * (a - 12) + 1)
            scores(15)
            sums(14)
            sums(15)
            rec(1)
            for a in range(8, 16):
                bc_probs(a)

        # ================= Phase C: fp8 DoubleRow O-proj + base add ======
        with tc.tile_pool(name="fin", bufs=3) as fin, \
             tc.tile_pool(name="ops", bufs=4, space="PSUM") as opp:
            # probsT col = 1024a + 512m + t, a = 2b+i
            pr_r = probsT_sb.rearrange("p (b i m t) -> p b i m t",
                                       b=8, i=2, m=MB)
            vwo_r = vwo_sb.rearrange("p (n b i c) -> p n b i c",
                                     n=KC, b=8, i=2)
            for ni in range(KC):
                bt = fin.tile([128, TC], bf16, tag="bt")
                nc.sync.dma_start(bt[:], baseT[128 * ni:128 * (ni + 1), :])
                osb = fin.tile([128, TC], f32, tag="osb")
                for m in range(MB):
                    op = opp.tile([128, 512], f32, tag="op",
                                  name=f"op{ni}_{m}")
                    for b in range(8):
                        nc.tensor.matmul(
                            op[:], vwo_r[:, ni, b], pr_r[:, b, :, m, :],
                            start=(b == 0), stop=(b == 7), perf_mode=DR,
                        )
                    ts = slice(512 * m, 512 * (m + 1))
                    nc.vector.scalar_tensor_tensor(
                        osb[:, ts], op[:], OSCALE, bt[:, ts], MUL, ADD)
                nc.sync.dma_start(outT[128 * ni:128 * (ni + 1), :], osb[:])

    nc.compile()
    return nc


def _host_prep(hidden_states, base_output, Wq, Wk, Wv, Wo, adaption_prompt,
               adaption_gate, position_ids, tc_tokens=TC, ncores=NCORES):
    bf16 = ml_dtypes.bfloat16
    fp8 = ml_dtypes.float8_e4m3
    f32 = np.float32

    def to_fp8(a):
        return np.clip(a, -240.0, 240.0).astype(fp8)

    x = np.ascontiguousarray(np.asarray(hidden_states, f32).reshape(T, HID))
    base = np.asarray(base_output, f32).reshape(T, HID)
    pos = np.asarray(position_ids).reshape(T).astype(np.int64)

    inv = 1.0 / (ROPE_THETA ** (np.arange(0, D, 2, dtype=f32) / D))
    freqs = pos[:, None].astype(f32) * inv[None, :]          # [T, 64]
    emb = np.concatenate([freqs, freqs], axis=1)             # [T, 128]
    # QSCALE compensates the fp8 scaling of the Q projection inputs
    cos = (np.cos(emb) * QSCALE).astype(f32)
    sin = (np.sin(emb) * QSCALE).astype(f32)
    # sin arm pairs with the row-swapped KT: +sin (d<64), -sin (d>=64)
    sin_signed = sin.copy()
    sin_signed[:, D // 2:] *= -1.0

    gate = f32(np.asarray(adaption_gate).reshape(-1)[0])
    scale = f32(1.0 / np.sqrt(D))
    prompt = np.asarray(adaption_prompt, f32).reshape(L, HID)

    def tile_doublerow(A):
        # A [K, N] -> [N, K] tiled: [128n+p, 256b+128i+c] = A[256b+128i+p, 128n+c]
        K, N = A.shape
        return np.ascontiguousarray(
            A.reshape(K // 256, 2, 128, N // 128, 128).transpose(3, 2, 0, 1, 4)
             .reshape(N, K))

    def tile_dr_rhs(A):
        # A [HID, n] -> [128, KC*n], cols (k2, mc, i, m):
        # [p, k2*2n + mc*1024 + i*512 + m] = A[256k2+128i+p, 512mc+m]
        n = A.shape[1]
        return np.ascontiguousarray(
            A.reshape(KC // 2, 2, 128, n // 512, 512)
             .transpose(2, 0, 3, 1, 4).reshape(128, KC * n))

    WqT = tile_doublerow(np.asarray(Wq, f32).T * (scale * f32(S_Q)))
    WqT = to_fp8(WqT)

    # adapter K (host fold): K = prompt @ Wk.T, per head [KT | KT row-swapped]
    Kmat = prompt @ np.asarray(Wk, f32).T                    # [L, HID]
    KT = Kmat.reshape(L, H, D).transpose(2, 1, 0)            # [D, H, L]
    ktp = np.empty((128, H, 2, L), f32)
    ktp[:, :, 0, :] = KT * S_K
    ktp[:, :, 1, :] = np.roll(KT, -D // 2, axis=0) * S_K
    ktpT = to_fp8(ktp.reshape(128, H * 2 * L))

    # folded V@Wo (host): VWo[h] = (prompt @ Wv.T * gate)[:, h] @ Wo.T[h]
    V = (prompt @ np.asarray(Wv, f32).T) * gate              # [L, HID]
    V5 = V.reshape(L, H, D).transpose(1, 0, 2)               # [H, L, D]
    WoT5 = np.asarray(Wo, f32).T.reshape(H, D, HID)          # [H, D, HID]
    M2 = (V5 @ WoT5).reshape(HL, HID)                        # [(h,l), HID]
    vwoT = to_fp8(tile_doublerow(M2 * f32(S_VW)))

    u2 = np.zeros((128, 32), f32)
    u2[0:64, 15] = 1.0 / S_PR
    u2[64:128, 16] = 1.0 / S_PR
    z2 = np.zeros((16, 16 * 64), f32)
    for k in range(16):
        z2[k, 64 * k:64 * (k + 1)] = 1.0
    u2 = u2.astype(bf16)
    z2 = z2.astype(bf16)

    in_maps = []
    for c in range(ncores):
        lo = c * tc_tokens
        hi = lo + tc_tokens
        in_maps.append({
            "xT": tile_dr_rhs(to_fp8(x[lo:hi].T * f32(S_X))),
            "cosT": np.ascontiguousarray(cos[lo:hi].T).astype(bf16),
            "sinT": np.ascontiguousarray(sin_signed[lo:hi].T).astype(bf16),
            "wqT": WqT,
            "ktpT": ktpT,
            "vwoT": vwoT,
            "baseT": np.ascontiguousarray(base[lo:hi].T).astype(bf16),
            "u2T": u2,
            "z2T": z2,
        })
    return in_maps


def kernel(hidden_states, base_output, Wq, Wk, Wv, Wo, adaption_prompt,
           adaption_gate, position_ids):
    from concourse import bass_utils

    if "nc" not in _cache:
        _cache["nc"] = _build()
    nc = _cache["nc"]

    in_maps = _host_prep(hidden_states, base_output, Wq, Wk, Wv, Wo,
                         adaption_prompt, adaption_gate, position_ids)

    res = bass_utils.run_bass_kernel_spmd(nc, in_maps, core_ids=list(range(NCORES)))

    out = np.empty((T, HID), np.float32)
    for c in range(NCORES):
        out[c * TC:(c + 1) * TC] = res.results[c]["outT"].T
    return out.reshape(B, S, HID)


# revision 20
# speedup vs baseline: 1.8961x; 1.0019x over previous
"""Distributed Trainium2 Bass kernel for AdaptedAttention (LLaMA-Adapter style).

Sharding: pure data-parallel over the B*S = 8192 token axis (1024 tokens per
core across 8 NeuronCores).  The adapter attention only attends to the L=64
adapter slots, so there is no cross-token dependency; each core produces its
own slice of the output.

Weight folding (host, numpy): the adapter K and the product V@Wo depend only
on weight-type inputs (adaption_prompt, Wk, Wv, Wo, adaption_gate), so they
are folded on the host like the other weight transforms (gate into Wv, scale
into Wq, RoPE tables).  Folding V@Wo per head gives a [H*L=2048, HID] output
projection, halving the O-proj FLOPs (the contraction drops 4096 -> 2048) and
removing the per-head probs@V matmuls entirely.

Device pipeline per core (all big matmuls fp8e4m3 DoubleRow, K=256/instr):
  - per head pair a (heads 2a, 2a+1): qT = WqT^T @ xT (PSUM), RoPE arms
    qa=q*cos, qb=q*sin' as fp8 (rotate-half eliminated: scores contract over
    the head dim, so scores = KT^T qa + KTrowswap^T qb, one DoubleRow mm with
    [KT|KTs] as the two K-groups); both heads of a pair land in one
    [128,512] PSUM tile -> one Exp activation.
  - softmax sums batched: an indicator-matrix matmul accumulates per-head
    column sums of exp into a shared [16,512] PSUM tile (value 1/64 folds the
    fp8 probs scale); one reciprocal per 16-head group replaces 64 tiny ones.
  - reciprocal rows are partition-broadcast with a second indicator matmul
    (PE, ~0.2us) and multiplied into probs (fp8, scale 64).
  - O-proj: outT = (VWo^T probsT) * OSCALE + baseT via fp8 DoubleRow matmuls
    over the (head,slot) contraction.
All stages are software-pipelined (scores lag 1 pair, sums lag 2, probs of
group 0 interleave with the Qproj of group 1) so TensorE never waits on
Scalar/Vector.
"""

import numpy as np
import ml_dtypes

B, S, HID = 4, 2048, 4096
H, D, L = 32, 128, 64
NCORES = 8
T = B * S
TC = T // NCORES          # tokens per core (1024)
KC = HID // 128           # 32 contraction chunks over hidden dim
HL = H * L                # folded O-proj contraction (2048)
MB = TC // 512            # 512-token m-chunks (2)
ROPE_THETA = 10000.0

S_X = 16.0                # fp8 scale on xT
S_Q = 8192.0              # fp8 scale on WqT (1/sqrt(D) already folded)
S_P = 16.0                # fp8 scale on qa/qb (rope products)
S_K = 16.0                # fp8 scale on adapter KT
S_PR = 64.0               # fp8 scale on probs (folded via 1/64 indicator)
S_VW = 1024.0             # fp8 scale on VWo
QSCALE = S_P / (S_X * S_Q)   # folded into the cos/sin tables on host
ESCALE = 1.0 / (S_P * S_K)   # descale via the exp activation's scale arg
OSCALE = 1.0 / (S_PR * S_VW)
SWI = True                   # DoubleRowSwInterleave weight layout (fast LDW)

_cache = {}


def _build(tc_tokens=TC):
    """Builds the SPMD Bass graph (identical on all 8 cores)."""
    import concourse.tile as tile
    from concourse import bacc, mybir
    from contextlib import ExitStack

    bf16 = mybir.dt.bfloat16
    fp8 = mybir.dt.float8e4
    f32 = mybir.dt.float32
    MUL = mybir.AluOpType.mult
    ADD = mybir.AluOpType.add
    EXP = mybir.ActivationFunctionType.Exp
    DR = (mybir.MatmulPerfMode.DoubleRowSwInterleave if SWI
          else mybir.MatmulPerfMode.DoubleRow)

    assert tc_tokens == TC and MB == 2

    nc = bacc.Bacc(
        "TRN2",
        target_bir_lowering=False,
        debug=False,
        enable_asserts=False,
    )

    # Host-pretiled layouts (every DMA a large contiguous burst):
    #   xT    [128, KC*tc]   : DoubleRow rhs tiling (see tile_dr_rhs)
    #   wqT   [H*128, KC*128]: [128h+p, 256k2+128i+c] = (Wq.T*s)[256k2+128i+p, 128h+c]
    #   ktpT  [128, H*2*L]   : [p, 128h+64i+l] = KT_h[p,l] (i=0) / KT_h[(p+64)%128,l] (i=1)
    #   vwoT  [KC*128, HL]   : [128n+p, 256b+128i+c] = (VWo*s)[256b+128i+p, 128n+c]
    xT = nc.dram_tensor("xT", [128, KC * TC], fp8, kind="ExternalInput").ap()
    cosT = nc.dram_tensor("cosT", [D, TC], bf16, kind="ExternalInput").ap()
    sinT = nc.dram_tensor("sinT", [D, TC], bf16, kind="ExternalInput").ap()
    wqT = nc.dram_tensor("wqT", [H * 128, KC * 128], fp8, kind="ExternalInput").ap()
    ktpT = nc.dram_tensor("ktpT", [128, H * 2 * L], fp8, kind="ExternalInput").ap()
    vwoT = nc.dram_tensor("vwoT", [KC * 128, HL], fp8, kind="ExternalInput").ap()
    baseT = nc.dram_tensor("baseT", [HID, TC], bf16, kind="ExternalInput").ap()
    u2T = nc.dram_tensor("u2T", [128, 32], bf16, kind="ExternalInput").ap()
    z2T = nc.dram_tensor("z2T", [16, 16 * 64], bf16, kind="ExternalInput").ap()
    outT = nc.dram_tensor("outT", [HID, TC], f32, kind="ExternalOutput").ap()

    with tile.TileContext(nc) as tc, ExitStack() as ctx:
        const_pool = ctx.enter_context(tc.tile_pool(name="const", bufs=1))
        persist = ctx.enter_context(tc.tile_pool(name="persist", bufs=1))

        # ---- persistent SBUF residents ----
        xT_sb = persist.tile([128, KC * TC], fp8)
        cos_sb = persist.tile([128, TC], bf16)
        sin_sb = persist.tile([128, TC], bf16)
        ktp_sb = persist.tile([128, H * 2 * L], fp8)
        probsT_sb = persist.tile([128, 16 * 1024], fp8)   # [p, 1024a+512m+t]
        vwo_sb = persist.tile([128, KC * HL], fp8)        # [p, 2048ni+col]
        U2 = const_pool.tile([128, 32], bf16)
        Z2 = const_pool.tile([16, 16 * 64], bf16)

        # First Qproj matmul needs only xT chunk 0 + wq head 0: issue those
        # DMAs first (descriptor issue serializes at ~0.6us each).
        nc.sync.dma_start(xT_sb[:, 0:2048], xT[:, 0:2048])

        def late_loads():
            nc.sync.dma_start(cos_sb[:], cosT[:])
            nc.sync.dma_start(sin_sb[:], sinT[:])
            nc.sync.dma_start(ktp_sb[:], ktpT[:])
            nc.sync.dma_start(U2[:], u2T[:])
            nc.sync.dma_start(Z2[:], z2T[:])
            for k2 in range(1, KC // 2):
                cs = slice(2048 * k2, 2048 * (k2 + 1))
                nc.sync.dma_start(xT_sb[:, cs], xT[:, cs])

        xT_r = xT_sb.rearrange("p (k q i m) -> p k q i m", k=KC // 2, q=MB, i=2)
        ktp_r = ktp_sb.rearrange("p (h i l) -> p h i l", h=H, i=2)

        # ============ Phase B: Qproj + RoPE + scores + softmax ============
        with tc.tile_pool(name="wq", bufs=2) as wqp, \
             tc.tile_pool(name="rope", bufs=10) as rp, \
             tc.tile_pool(name="esb", bufs=34) as esbp, \
             tc.tile_pool(name="recs", bufs=4) as recp, \
             tc.tile_pool(name="qps", bufs=2, space="PSUM") as qpsp, \
             tc.tile_pool(name="scps", bufs=2, space="PSUM") as scp, \
             tc.tile_pool(name="sups", bufs=2, space="PSUM") as sup, \
             tc.tile_pool(name="bcps", bufs=2, space="PSUM") as bcp:

            qab_st, esb_st, sums_st, rec_st, wq_st = {}, {}, {}, {}, {}

            def wq_load(j):
                wq_sb = wqp.tile([128, KC * 128], fp8, tag="wq")
                nc.sync.dma_start(wq_sb[:], wqT[128 * j:128 * (j + 1), :])
                wq_st[j] = wq_sb

            def qproj(a):
                for j in (2 * a, 2 * a + 1):
                    wq_sb = wq_st.pop(j)
                    wq_r = wq_sb.rearrange("p (k i c) -> p k i c",
                                           k=KC // 2, i=2)
                    for m in range(MB):
                        qp = qpsp.tile([128, 512], f32, tag="qp",
                                       name=f"qp{j}_{m}")
                        for k2 in range(KC // 2):
                            lhsT = (wq_sb[:, 256 * k2:256 * (k2 + 1)]
                                    if SWI else wq_r[:, k2])
                            nc.tensor.matmul(
                                qp[:], lhsT, xT_r[:, k2, m],
                                start=(k2 == 0), stop=(k2 == KC // 2 - 1),
                                perf_mode=DR,
                            )
                        ms = slice(512 * m, 512 * (m + 1))
                        qab = rp.tile([128, 1024], fp8, tag="qab",
                                      name=f"qab{j}_{m}")
                        nc.vector.tensor_tensor(qab[:, 0:512], qp[:],
                                                cos_sb[:, ms], MUL)
                        nc.vector.tensor_tensor(qab[:, 512:1024], qp[:],
                                                sin_sb[:, ms], MUL)
                        qab_st[(j, m)] = qab

            def scores(a):
                # DoubleRow can't col-tile (XBUS budget), so the two RoPE
                # arms accumulate as two plain fp8 matmuls per head; the two
                # heads of a pair land in col-groups 0-63 / 64-127.
                for m in range(MB):
                    psc = scp.tile([128, 512], f32, tag="sc", name=f"sc{a}_{m}")
                    for i, j in enumerate((2 * a, 2 * a + 1)):
                        qab = qab_st.pop((j, m))
                        for arm in range(2):
                            nc.tensor.matmul(
                                psc[64 * i:64 * (i + 1), :],
                                ktp_r[:, j, arm, :],
                                qab[:, 512 * arm:512 * (arm + 1)],
                                start=(arm == 0), stop=(arm == 1),
                            )
                    esb = esbp.tile([128, 512], bf16, tag="esb",
                                    name=f"esb{a}_{m}")
                    nc.scalar.activation(esb[:], psc[:], EXP, scale=ESCALE)
                    esb_st[(a, m)] = esb

            def sums(a):
                g, al = a // 8, a % 8
                for m in range(MB):
                    if al == 0:
                        sums_st[(g, m)] = sup.tile([16, 512], f32, tag="sums",
                                                   name=f"su{g}_{m}")
                    nc.tensor.matmul(
                        sums_st[(g, m)][:], U2[:, 15 - 2 * al:31 - 2 * al],
                        esb_st[(a, m)][:], start=(al == 0), stop=(al == 7),
                    )

            def rec(g):
                for m in range(MB):
                    rc = recp.tile([16, 512], bf16, tag="rec",
                                   name=f"rec{g}_{m}")
                    with nc.allow_low_precision(reason="bf16 softmax weights"):
                        nc.vector.reciprocal(rc[:], sums_st[(g, m)][:])
                    rec_st[(g, m)] = rc

            def bc_probs(a):
                g, al = a // 8, a % 8
                for m in range(MB):
                    pb = bcp.tile([128, 512], f32, tag="bcp", name=f"bc{a}_{m}")
                    nc.tensor.matmul(pb[:], Z2[:, 128 * al:128 * (al + 1)],
                                     rec_st[(g, m)][:], start=True, stop=True)
                    ps = slice(1024 * a + 512 * m, 1024 * a + 512 * (m + 1))
                    nc.vector.tensor_tensor(probsT_sb[:, ps],
                                            esb_st.pop((a, m))[:], pb[:], MUL)

            wq_load(0)
            wq_load(1)
            late_loads()
            for a in range(16):
                qproj(a)
                if a < 15:      # prefetch next pair's weights
                    wq_load(2 * a + 2)
                    wq_load(2 * a + 3)
                # stream the resident O-proj weights in during phase B
                for ni in (2 * a, 2 * a + 1):
                    nc.sync.dma_start(
                        vwo_sb[:, HL * ni:HL * (ni + 1)],
                        vwoT[128 * ni:128 * (ni + 1), :])
                if a >= 1:
                    scores(a - 1)
                if a == 10:
                    rec(0)      # before sums(8) so group-0 PSUM slots free
                if a >= 2:
                    sums(a - 2)
                if a >= 12:
                    bc_probs(2 * (a - 12))
                    bc_probs(2 * (a - 12) + 1)
            scores(15)
            sums(14)
            sums(15)
            rec(1)
            for a in range(8, 16):
                bc_probs(a)

        # ================= Phase C: fp8 DoubleRow O-proj + base add ======
        with tc.tile_pool(name="fin", bufs=3) as fin, \
             tc.tile_pool(name="ops", bufs=4, space="PSUM") as opp:
            # probsT col = 1024a + 512m + t, a = 2b+i
            pr_r = probsT_sb.rearrange("p (b i m t) -> p b i m t",
                                       b=8, i=2, m=MB)
            vwo_r = vwo_sb.rearrange("p (n b i c) -> p n b i c",
                                     n=KC, b=8, i=2)
            for ni in range(KC):
                bt = fin.tile([128, TC], bf16, tag="bt")
                nc.sync.dma_start(bt[:], baseT[128 * ni:128 * (ni + 1), :])
                osb = fin.tile([128, TC], f32, tag="osb")
                for m in range(MB):
                    op = opp.tile([128, 512], f32, tag="op",
                                  name=f"op{ni}_{m}")
                    for b in range(8):
                        lhsT = (vwo_sb[:, HL * ni + 256 * b:
                                       HL * ni + 256 * (b + 1)]
                                if SWI else vwo_r[:, ni, b])
                        nc.tensor.matmul(
                            op[:], lhsT, pr_r[:, b, :, m, :],
                            start=(b == 0), stop=(b == 7), perf_mode=DR,
                        )
                    ts = slice(512 * m, 512 * (m + 1))
                    nc.vector.scalar_tensor_tensor(
                        osb[:, ts], op[:], OSCALE, bt[:, ts], MUL, ADD)
                nc.sync.dma_start(outT[128 * ni:128 * (ni + 1), :], osb[:])

    nc.compile()
    return nc


def _host_prep(hidden_states, base_output, Wq, Wk, Wv, Wo, adaption_prompt,
               adaption_gate, position_ids, tc_tokens=TC, ncores=NCORES):
    bf16 = ml_dtypes.bfloat16
    fp8 = ml_dtypes.float8_e4m3
    f32 = np.float32

    def to_fp8(a):
        return np.clip(a, -240.0, 240.0).astype(fp8)

    x = np.ascontiguousarray(np.asarray(hidden_states, f32).reshape(T, HID))
    base = np.asarray(base_output, f32).reshape(T, HID)
    pos = np.asarray(position_ids).reshape(T).astype(np.int64)

    inv = 1.0 / (ROPE_THETA ** (np.arange(0, D, 2, dtype=f32) / D))
    freqs = pos[:, None].astype(f32) * inv[None, :]          # [T, 64]
    emb = np.concatenate([freqs, freqs], axis=1)             # [T, 128]
    # QSCALE compensates the fp8 scaling of the Q projection inputs
    cos = (np.cos(emb) * QSCALE).astype(f32)
    sin = (np.sin(emb) * QSCALE).astype(f32)
    # sin arm pairs with the row-swapped KT: +sin (d<64), -sin (d>=64)
    sin_signed = sin.copy()
    sin_signed[:, D // 2:] *= -1.0

    gate = f32(np.asarray(adaption_gate).reshape(-1)[0])
    scale = f32(1.0 / np.sqrt(D))
    prompt = np.asarray(adaption_prompt, f32).reshape(L, HID)

    def tile_doublerow(A):
        # A [K, N] -> [N, K] tiled.
        # DoubleRow:       [128n+p, 256b+128i+c]    = A[256b+128i+p, 128n+c]
        # SwInterleave:    [128n+p, 256b+2(127-c)+i] = A[256b+128i+p, 128n+c]
        K, N = A.shape
        t = A.reshape(K // 256, 2, 128, N // 128, 128).transpose(3, 2, 0, 1, 4)
        if SWI:                       # (n, p, b, i, c) -> (n, p, b, 127-c, i)
            t = t[..., ::-1].transpose(0, 1, 2, 4, 3)
        return np.ascontiguousarray(t.reshape(N, K))

    def tile_dr_rhs(A):
        # A [HID, n] -> [128, KC*n], cols (k2, mc, i, m):
        # [p, k2*2n + mc*1024 + i*512 + m] = A[256k2+128i+p, 512mc+m]
        n = A.shape[1]
        return np.ascontiguousarray(
            A.reshape(KC // 2, 2, 128, n // 512, 512)
             .transpose(2, 0, 3, 1, 4).reshape(128, KC * n))

    WqT = tile_doublerow(np.asarray(Wq, f32).T * (scale * f32(S_Q)))
    WqT = to_fp8(WqT)

    # adapter K (host fold): K = prompt @ Wk.T, per head [KT | KT row-swapped]
    Kmat = prompt @ np.asarray(Wk, f32).T                    # [L, HID]
    KT = Kmat.reshape(L, H, D).transpose(2, 1, 0)            # [D, H, L]
    ktp = np.empty((128, H, 2, L), f32)
    ktp[:, :, 0, :] = KT * S_K
    ktp[:, :, 1, :] = np.roll(KT, -D // 2, axis=0) * S_K
    ktpT = to_fp8(ktp.reshape(128, H * 2 * L))

    # folded V@Wo (host): VWo[h] = (prompt @ Wv.T * gate)[:, h] @ Wo.T[h]
    V = (prompt @ np.asarray(Wv, f32).T) * gate              # [L, HID]
    V5 = V.reshape(L, H, D).transpose(1, 0, 2)               # [H, L, D]
    WoT5 = np.asarray(Wo, f32).T.reshape(H, D, HID)          # [H, D, HID]
    M2 = (V5 @ WoT5).reshape(HL, HID)                        # [(h,l), HID]
    vwoT = to_fp8(tile_doublerow(M2 * f32(S_VW)))

    u2 = np.zeros((128, 32), f32)
    u2[0:64, 15] = 1.0 / S_PR
    u2[64:128, 16] = 1.0 / S_PR
    z2 = np.zeros((16, 16 * 64), f32)
    for k in range(16):
        z2[k, 64 * k:64 * (k + 1)] = 1.0
    u2 = u2.astype(bf16)
    z2 = z2.astype(bf16)

    in_maps = []
    for c in range(ncores):
        lo = c * tc_tokens
        hi = lo + tc_tokens
        in_maps.append({
            "xT": tile_dr_rhs(to_fp8(x[lo:hi].T * f32(S_X))),
            "cosT": np.ascontiguousarray(cos[lo:hi].T).astype(bf16),
            "sinT": np.ascontiguousarray(sin_signed[lo:hi].T).astype(bf16),
            "wqT": WqT,
            "ktpT": ktpT,
            "vwoT": vwoT,
            "baseT": np.ascontiguousarray(base[lo:hi].T).astype(bf16),
            "u2T": u2,
            "z2T": z2,
        })
    return in_maps


def kernel(hidden_states, base_output, Wq, Wk, Wv, Wo, adaption_prompt,
           adaption_gate, position_ids):
    from concourse import bass_utils

    if "nc" not in _cache:
        _cache["nc"] = _build()
    nc = _cache["nc"]

    in_maps = _host_prep(hidden_states, base_output, Wq, Wk, Wv, Wo,
                         adaption_prompt, adaption_gate, position_ids)

    res = bass_utils.run_bass_kernel_spmd(nc, in_maps, core_ids=list(range(NCORES)))

    out = np.empty((T, HID), np.float32)
    for c in range(NCORES):
        out[c * TC:(c + 1) * TC] = res.results[c]["outT"].T
    return out.reshape(B, S, HID)


# revision 23
# speedup vs baseline: 1.8993x; 1.0017x over previous
"""Distributed Trainium2 Bass kernel for AdaptedAttention (LLaMA-Adapter style).

Sharding: pure data-parallel over the B*S = 8192 token axis (1024 tokens per
core across 8 NeuronCores).  The adapter attention only attends to the L=64
adapter slots, so there is no cross-token dependency; each core produces its
own slice of the output.

Weight folding (host, numpy): the adapter K and the product V@Wo depend only
on weight-type inputs (adaption_prompt, Wk, Wv, Wo, adaption_gate), so they
are folded on the host like the other weight transforms (gate into Wv, scale
into Wq, RoPE tables).  Folding V@Wo per head gives a [H*L=2048, HID] output
projection, halving the O-proj FLOPs (the contraction drops 4096 -> 2048) and
removing the per-head probs@V matmuls entirely.

Device pipeline per core (all big matmuls fp8e4m3 DoubleRow, K=256/instr):
  - per head pair a (heads 2a, 2a+1): qT = WqT^T @ xT (PSUM), RoPE arms
    qa=q*cos, qb=q*sin' as fp8 (rotate-half eliminated: scores contract over
    the head dim, so scores = KT^T qa + KTrowswap^T qb, one DoubleRow mm with
    [KT|KTs] as the two K-groups); both heads of a pair land in one
    [128,512] PSUM tile -> one Exp activation.
  - softmax sums batched: an indicator-matrix matmul accumulates per-head
    column sums of exp into a shared [16,512] PSUM tile (value 1/64 folds the
    fp8 probs scale); one reciprocal per 16-head group replaces 64 tiny ones.
  - reciprocal rows are partition-broadcast with a second indicator matmul
    (PE, ~0.2us) and multiplied into probs (fp8, scale 64).
  - O-proj: outT = (VWo^T probsT) * OSCALE + baseT via fp8 DoubleRow matmuls
    over the (head,slot) contraction.
All stages are software-pipelined (scores lag 1 pair, sums lag 2, probs of
group 0 interleave with the Qproj of group 1) so TensorE never waits on
Scalar/Vector.
"""

import numpy as np
import ml_dtypes

B, S, HID = 4, 2048, 4096
H, D, L = 32, 128, 64
NCORES = 8
T = B * S
TC = T // NCORES          # tokens per core (1024)
KC = HID // 128           # 32 contraction chunks over hidden dim
HL = H * L                # folded O-proj contraction (2048)
MB = TC // 512            # 512-token m-chunks (2)
ROPE_THETA = 10000.0

S_X = 16.0                # fp8 scale on xT
S_Q = 8192.0              # fp8 scale on WqT (1/sqrt(D) already folded)
S_P = 16.0                # fp8 scale on qa/qb (rope products)
S_K = 16.0                # fp8 scale on adapter KT
S_PR = 64.0               # fp8 scale on probs (folded via 1/64 indicator)
S_VW = 1024.0             # fp8 scale on VWo
QSCALE = S_P / (S_X * S_Q)   # folded into the cos/sin tables on host
ESCALE = 1.0 / (S_P * S_K)   # descale via the exp activation's scale arg
OSCALE = 1.0 / (S_PR * S_VW)
SWI = True                   # DoubleRowSwInterleave weight layout (fast LDW)

_cache = {}


def _build(tc_tokens=TC):
    """Builds the SPMD Bass graph (identical on all 8 cores)."""
    import concourse.tile as tile
    from concourse import bacc, mybir
    from contextlib import ExitStack

    bf16 = mybir.dt.bfloat16
    fp8 = mybir.dt.float8e4
    f32 = mybir.dt.float32
    MUL = mybir.AluOpType.mult
    ADD = mybir.AluOpType.add
    EXP = mybir.ActivationFunctionType.Exp
    DR = (mybir.MatmulPerfMode.DoubleRowSwInterleave if SWI
          else mybir.MatmulPerfMode.DoubleRow)

    assert tc_tokens == TC and MB == 2

    nc = bacc.Bacc(
        "TRN2",
        target_bir_lowering=False,
        debug=False,
        enable_asserts=False,
    )

    # Host-pretiled layouts (every DMA a large contiguous burst):
    #   xT    [128, KC*tc]   : DoubleRow rhs tiling (see tile_dr_rhs)
    #   wqT   [H*128, KC*128]: [128h+p, 256k2+128i+c] = (Wq.T*s)[256k2+128i+p, 128h+c]
    #   ktpT  [128, H*2*L]   : [p, 128h+64i+l] = KT_h[p,l] (i=0) / KT_h[(p+64)%128,l] (i=1)
    #   vwoT  [KC*128, HL]   : [128n+p, 256b+128i+c] = (VWo*s)[256b+128i+p, 128n+c]
    xT = nc.dram_tensor("xT", [128, KC * TC], fp8, kind="ExternalInput").ap()
    cosT = nc.dram_tensor("cosT", [D, TC], bf16, kind="ExternalInput").ap()
    sinT = nc.dram_tensor("sinT", [D, TC], bf16, kind="ExternalInput").ap()
    wqT = nc.dram_tensor("wqT", [H * 128, KC * 128], fp8, kind="ExternalInput").ap()
    ktpT = nc.dram_tensor("ktpT", [128, H * 2 * L], fp8, kind="ExternalInput").ap()
    vwoT = nc.dram_tensor("vwoT", [KC * 128, HL], fp8, kind="ExternalInput").ap()
    baseT = nc.dram_tensor("baseT", [HID, TC], bf16, kind="ExternalInput").ap()
    u2T = nc.dram_tensor("u2T", [128, 32], bf16, kind="ExternalInput").ap()
    z2T = nc.dram_tensor("z2T", [16, 16 * 64], bf16, kind="ExternalInput").ap()
    outT = nc.dram_tensor("outT", [HID, TC], f32, kind="ExternalOutput").ap()

    with tile.TileContext(nc) as tc, ExitStack() as ctx:
        const_pool = ctx.enter_context(tc.tile_pool(name="const", bufs=1))
        persist = ctx.enter_context(tc.tile_pool(name="persist", bufs=1))

        # ---- persistent SBUF residents ----
        xT_sb = persist.tile([128, KC * TC], fp8)
        cos_sb = persist.tile([128, TC], bf16)
        sin_sb = persist.tile([128, TC], bf16)
        ktp_sb = persist.tile([128, H * 2 * L], fp8)
        probsT_sb = persist.tile([128, 16 * 1024], fp8)   # [p, 1024a+512m+t]
        vwo_sb = persist.tile([128, KC * HL], fp8)        # [p, 2048ni+col]
        U2 = const_pool.tile([128, 32], bf16)
        Z2 = const_pool.tile([16, 16 * 64], bf16)

        def late_loads():
            nc.sync.dma_start(cos_sb[:], cosT[:])
            nc.sync.dma_start(sin_sb[:], sinT[:])
            nc.sync.dma_start(ktp_sb[:], ktpT[:])
            nc.sync.dma_start(U2[:], u2T[:])
            nc.sync.dma_start(Z2[:], z2T[:])
            for k2 in range(1, KC // 2):
                cs = slice(2048 * k2, 2048 * (k2 + 1))
                nc.sync.dma_start(xT_sb[:, cs], xT[:, cs])

        xT_r = xT_sb.rearrange("p (k q i m) -> p k q i m", k=KC // 2, q=MB, i=2)
        ktp_r = ktp_sb.rearrange("p (h i l) -> p h i l", h=H, i=2)

        # ============ Phase B: Qproj + RoPE + scores + softmax ============
        with tc.tile_pool(name="wq", bufs=2) as wqp, \
             tc.tile_pool(name="rope", bufs=10) as rp, \
             tc.tile_pool(name="esb", bufs=34) as esbp, \
             tc.tile_pool(name="recs", bufs=4) as recp, \
             tc.tile_pool(name="fin", bufs=3) as fin, \
             tc.tile_pool(name="sups", bufs=2, space="PSUM") as sup, \
             tc.tile_pool(name="bcps", bufs=2, space="PSUM") as bcp:

            qab_st, esb_st, sums_st, rec_st, wq_st = {}, {}, {}, {}, {}

            def wq_load(j):
                wq_sb = wqp.tile([128, KC * 128], fp8, tag="wq")
                nc.sync.dma_start(wq_sb[:], wqT[128 * j:128 * (j + 1), :])
                wq_st[j] = wq_sb

            def qproj(a):
                for j in (2 * a, 2 * a + 1):
                    wq_sb = wq_st.pop(j)
                    wq_r = wq_sb.rearrange("p (k i c) -> p k i c",
                                           k=KC // 2, i=2)
                    for m in range(MB):
                        qp = qpsp.tile([128, 512], f32, tag="qp",
                                       name=f"qp{j}_{m}")
                        for k2 in range(KC // 2):
                            lhsT = (wq_sb[:, 256 * k2:256 * (k2 + 1)]
                                    if SWI else wq_r[:, k2])
                            nc.tensor.matmul(
                                qp[:], lhsT, xT_r[:, k2, m],
                                start=(k2 == 0), stop=(k2 == KC // 2 - 1),
                                perf_mode=DR,
                            )
                        ms = slice(512 * m, 512 * (m + 1))
                        qab = rp.tile([128, 1024], fp8, tag="qab",
                                      name=f"qab{j}_{m}")
                        nc.vector.tensor_tensor(qab[:, 0:512], qp[:],
                                                cos_sb[:, ms], MUL)
                        nc.vector.tensor_tensor(qab[:, 512:1024], qp[:],
                                                sin_sb[:, ms], MUL)
                        qab_st[(j, m)] = qab

            def scores(a):
                # DoubleRow can't col-tile (XBUS budget), so the two RoPE
                # arms accumulate as two plain fp8 matmuls per head; the two
                # heads of a pair land in col-groups 0-63 / 64-127.
                for m in range(MB):
                    psc = scp.tile([128, 512], f32, tag="sc", name=f"sc{a}_{m}")
                    for i, j in enumerate((2 * a, 2 * a + 1)):
                        qab = qab_st.pop((j, m))
                        for arm in range(2):
                            nc.tensor.matmul(
                                psc[64 * i:64 * (i + 1), :],
                                ktp_r[:, j, arm, :],
                                qab[:, 512 * arm:512 * (arm + 1)],
                                start=(arm == 0), stop=(arm == 1),
                            )
                    esb = esbp.tile([128, 512], bf16, tag="esb",
                                    name=f"esb{a}_{m}")
                    nc.scalar.activation(esb[:], psc[:], EXP, scale=ESCALE)
                    esb_st[(a, m)] = esb

            def sums(a):
                g, al = a // 8, a % 8
                for m in range(MB):
                    if al == 0:
                        sums_st[(g, m)] = sup.tile([16, 512], f32, tag="sums",
                                                   name=f"su{g}_{m}")
                    nc.tensor.matmul(
                        sums_st[(g, m)][:], U2[:, 15 - 2 * al:31 - 2 * al],
                        esb_st[(a, m)][:], start=(al == 0), stop=(al == 7),
                    )

            def rec(g):
                for m in range(MB):
                    rc = recp.tile([16, 512], bf16, tag="rec",
                                   name=f"rec{g}_{m}")
                    with nc.allow_low_precision(reason="bf16 softmax weights"):
                        nc.vector.reciprocal(rc[:], sums_st[(g, m)][:])
                    rec_st[(g, m)] = rc

            def bc_probs(a):
                g, al = a // 8, a % 8
                for m in range(MB):
                    pb = bcp.tile([128, 512], f32, tag="bcp", name=f"bc{a}_{m}")
                    nc.tensor.matmul(pb[:], Z2[:, 128 * al:128 * (al + 1)],
                                     rec_st[(g, m)][:], start=True, stop=True)
                    ps = slice(1024 * a + 512 * m, 1024 * a + 512 * (m + 1))
                    nc.vector.tensor_tensor(probsT_sb[:, ps],
                                            esb_st.pop((a, m))[:], pb[:], MUL)

            # Qproj/scores PSUM pools nest so their 4 banks free up for the
            # O-proj tiles that overlap the group-1 softmax tail.
            with tc.tile_pool(name="qps", bufs=2, space="PSUM") as qpsp, \
                 tc.tile_pool(name="scps", bufs=2, space="PSUM") as scp:
                wq_load(0)
                nc.sync.dma_start(xT_sb[:, 0:2048], xT[:, 0:2048])
                wq_load(1)
                late_loads()
                for a in range(16):
                    qproj(a)
                    if a < 15:      # prefetch next pair's weights
                        wq_load(2 * a + 2)
                        wq_load(2 * a + 3)
                    # stream the resident O-proj weights in during phase B
                    for ni in (2 * a, 2 * a + 1):
                        nc.sync.dma_start(
                            vwo_sb[:, HL * ni:HL * (ni + 1)],
                            vwoT[128 * ni:128 * (ni + 1), :])
                    if a >= 1:
                        scores(a - 1)
                    if a == 10:
                        rec(0)  # before sums(8) so group-0 PSUM slots free
                    if a >= 2:
                        sums(a - 2)
                    if a >= 12:
                        bc_probs(2 * (a - 12))
                        bc_probs(2 * (a - 12) + 1)
                scores(15)
                sums(14)

            # ============ Phase C: fp8 DoubleRow O-proj + base add ========
            # ni 0/1 accumulate their group-0 chunks (b 0-3) while the
            # group-1 reciprocal/broadcast/probs tail is still running.
            with tc.tile_pool(name="ops", bufs=4, space="PSUM") as opp:
                pr_r = probsT_sb.rearrange("p (b i m t) -> p b i m t",
                                           b=8, i=2, m=MB)
                vwo_r = vwo_sb.rearrange("p (n b i c) -> p n b i c",
                                         n=KC, b=8, i=2)

                def omm(op, ni, m, b, start, stop):
                    lhsT = (vwo_sb[:, HL * ni + 256 * b:HL * ni + 256 * (b + 1)]
                            if SWI else vwo_r[:, ni, b])
                    nc.tensor.matmul(op[:], lhsT, pr_r[:, b, :, m, :],
                                     start=start, stop=stop, perf_mode=DR)

                def ofin(ni, osb, op, m, bt):
                    ts = slice(512 * m, 512 * (m + 1))
                    nc.vector.scalar_tensor_tensor(
                        osb[:, ts], op[:], OSCALE, bt[:, ts], MUL, ADD)

                op_st, bt_st = {}, {}
                for ni in (0, 1):
                    bt = fin.tile([128, TC], bf16, tag="bt", name=f"bt{ni}")
                    nc.sync.dma_start(bt[:], baseT[128 * ni:128 * (ni + 1), :])
                    bt_st[ni] = bt
                    for m in range(MB):
                        op = opp.tile([128, 512], f32, tag="op",
                                      name=f"op{ni}_{m}")
                        op_st[(ni, m)] = op
                        for b in range(4):
                            omm(op, ni, m, b, b == 0, False)
                sums(15)
                rec(1)
                for a in range(8, 16):
                    bc_probs(a)
                for ni in (0, 1):
                    osb = fin.tile([128, TC], f32, tag="osb")
                    bt = bt_st.pop(ni)
                    for m in range(MB):
                        op = op_st.pop((ni, m))
                        for b in range(4, 8):
                            omm(op, ni, m, b, False, b == 7)
                        ofin(ni, osb, op, m, bt)
                    nc.sync.dma_start(outT[128 * ni:128 * (ni + 1), :], osb[:])

                for ni in range(2, KC):
                    bt = fin.tile([128, TC], bf16, tag="bt")
                    nc.sync.dma_start(bt[:], baseT[128 * ni:128 * (ni + 1), :])
                    osb = fin.tile([128, TC], f32, tag="osb")
                    for m in range(MB):
                        op = opp.tile([128, 512], f32, tag="op",
                                      name=f"op{ni}_{m}")
                        for b in range(8):
                            omm(op, ni, m, b, b == 0, b == 7)
                        ofin(ni, osb, op, m, bt)
                    nc.sync.dma_start(
                        outT[128 * ni:128 * (ni + 1), :], osb[:])

    nc.compile()
    return nc


def _host_prep(hidden_states, base_output, Wq, Wk, Wv, Wo, adaption_prompt,
               adaption_gate, position_ids, tc_tokens=TC, ncores=NCORES):
    bf16 = ml_dtypes.bfloat16
    fp8 = ml_dtypes.float8_e4m3
    f32 = np.float32

    def to_fp8(a):
        return np.clip(a, -240.0, 240.0).astype(fp8)

    x = np.ascontiguousarray(np.asarray(hidden_states, f32).reshape(T, HID))
    base = np.asarray(base_output, f32).reshape(T, HID)
    pos = np.asarray(position_ids).reshape(T).astype(np.int64)

    inv = 1.0 / (ROPE_THETA ** (np.arange(0, D, 2, dtype=f32) / D))
    freqs = pos[:, None].astype(f32) * inv[None, :]          # [T, 64]
    emb = np.concatenate([freqs, freqs], axis=1)             # [T, 128]
    # QSCALE compensates the fp8 scaling of the Q projection inputs
    cos = (np.cos(emb) * QSCALE).astype(f32)
    sin = (np.sin(emb) * QSCALE).astype(f32)
    # sin arm pairs with the row-swapped KT: +sin (d<64), -sin (d>=64)
    sin_signed = sin.copy()
    sin_signed[:, D // 2:] *= -1.0

    gate = f32(np.asarray(adaption_gate).reshape(-1)[0])
    scale = f32(1.0 / np.sqrt(D))
    prompt = np.asarray(adaption_prompt, f32).reshape(L, HID)

    def tile_doublerow(A):
        # A [K, N] -> [N, K] tiled.
        # DoubleRow:       [128n+p, 256b+128i+c]    = A[256b+128i+p, 128n+c]
        # SwInterleave:    [128n+p, 256b+2(127-c)+i] = A[256b+128i+p, 128n+c]
        K, N = A.shape
        t = A.reshape(K // 256, 2, 128, N // 128, 128).transpose(3, 2, 0, 1, 4)
        if SWI:                       # (n, p, b, i, c) -> (n, p, b, 127-c, i)
            t = t[..., ::-1].transpose(0, 1, 2, 4, 3)
        return np.ascontiguousarray(t.reshape(N, K))

    def tile_dr_rhs(A):
        # A [HID, n] -> [128, KC*n], cols (k2, mc, i, m):
        # [p, k2*2n + mc*1024 + i*512 + m] = A[256k2+128i+p, 512mc+m]
        n = A.shape[1]
        return np.ascontiguousarray(
            A.reshape(KC // 2, 2, 128, n // 512, 512)
             .transpose(2, 0, 3, 1, 4).reshape(128, KC * n))

    WqT = tile_doublerow(np.asarray(Wq, f32).T * (scale * f32(S_Q)))
    WqT = to_fp8(WqT)

    # adapter K (host fold): K = prompt @ Wk.T, per head [KT | KT row-swapped]
    Kmat = prompt @ np.asarray(Wk, f32).T                    # [L, HID]
    KT = Kmat.reshape(L, H, D).transpose(2, 1, 0)            # [D, H, L]
    ktp = np.empty((128, H, 2, L), f32)
    ktp[:, :, 0, :] = KT * S_K
    ktp[:, :, 1, :] = np.roll(KT, -D // 2, axis=0) * S_K
    ktpT = to_fp8(ktp.reshape(128, H * 2 * L))

    # folded V@Wo (host): VWo[h] = (prompt @ Wv.T * gate)[:, h] @ Wo.T[h]
    V = (prompt @ np.asarray(Wv, f32).T) * gate              # [L, HID]
    V5 = V.reshape(L, H, D).transpose(1, 0, 2)               # [H, L, D]
    WoT5 = np.asarray(Wo, f32).T.reshape(H, D, HID)          # [H, D, HID]
    M2 = (V5 @ WoT5).reshape(HL, HID)                        # [(h,l), HID]
    vwoT = to_fp8(tile_doublerow(M2 * f32(S_VW)))

    u2 = np.zeros((128, 32), f32)
    u2[0:64, 15] = 1.0 / S_PR
    u2[64:128, 16] = 1.0 / S_PR
    z2 = np.zeros((16, 16 * 64), f32)
    for k in range(16):
        z2[k, 64 * k:64 * (k + 1)] = 1.0
    u2 = u2.astype(bf16)
    z2 = z2.astype(bf16)

    in_maps = []
    for c in range(ncores):
        lo = c * tc_tokens
        hi = lo + tc_tokens
        in_maps.append({
            "xT": tile_dr_rhs(to_fp8(x[lo:hi].T * f32(S_X))),
            "cosT": np.ascontiguousarray(cos[lo:hi].T).astype(bf16),
            "sinT": np.ascontiguousarray(sin_signed[lo:hi].T).astype(bf16),
            "wqT": WqT,
            "ktpT": ktpT,
            "vwoT": vwoT,
            "baseT": np.ascontiguousarray(base[lo:hi].T).astype(bf16),
            "u2T": u2,
            "z2T": z2,
        })
    return in_maps


def kernel(hidden_states, base_output, Wq, Wk, Wv, Wo, adaption_prompt,
           adaption_gate, position_ids):
    from concourse import bass_utils

    if "nc" not in _cache:
        _cache["nc"] = _build()
    nc = _cache["nc"]

    in_maps = _host_prep(hidden_states, base_output, Wq, Wk, Wv, Wo,
                         adaption_prompt, adaption_gate, position_ids)

    res = bass_utils.run_bass_kernel_spmd(nc, in_maps, core_ids=list(range(NCORES)))

    out = np.empty((T, HID), np.float32)
    for c in range(NCORES):
        out[c * TC:(c + 1) * TC] = res.results[c]["outT"].T
    return out.reshape(B, S, HID)


# revision 34
# speedup vs baseline: 1.9261x; 1.0141x over previous
"""Distributed Trainium2 Bass kernel for AdaptedAttention (LLaMA-Adapter style).

Sharding: pure data-parallel over the B*S = 8192 token axis (1024 tokens per
core across 8 NeuronCores).  The adapter attention only attends to the L=64
adapter slots, so there is no cross-token dependency; each core produces its
own slice of the output.

Weight folding (host, numpy): the adapter K and the product V@Wo depend only
on weight-type inputs (adaption_prompt, Wk, Wv, Wo, adaption_gate), so they
are folded on the host like the other weight transforms (gate into Wv, scale
into Wq, RoPE tables).  Folding V@Wo per head gives a [H*L=2048, HID] output
projection, halving the O-proj FLOPs (the contraction drops 4096 -> 2048) and
removing the per-head probs@V matmuls entirely.

Device pipeline per core (all big matmuls fp8e4m3 DoubleRow, K=256/instr):
  - per head pair a (heads 2a, 2a+1): qT = WqT^T @ xT (PSUM), RoPE arms
    qa=q*cos, qb=q*sin' as fp8 (rotate-half eliminated: scores contract over
    the head dim, so scores = KT^T qa + KTrowswap^T qb, one DoubleRow mm with
    [KT|KTs] as the two K-groups); both heads of a pair land in one
    [128,512] PSUM tile -> one Exp activation.
  - softmax sums batched: an indicator-matrix matmul accumulates per-head
    column sums of exp into a shared [16,512] PSUM tile (value 1/64 folds the
    fp8 probs scale); one reciprocal per 16-head group replaces 64 tiny ones.
  - reciprocal rows are partition-broadcast with a second indicator matmul
    (PE, ~0.2us) and multiplied into probs (fp8, scale 64).
  - O-proj: outT = (VWo^T probsT) * OSCALE + baseT via fp8 DoubleRow matmuls
    over the (head,slot) contraction.
All stages are software-pipelined (scores lag 1 pair, sums lag 2, probs of
group 0 interleave with the Qproj of group 1) so TensorE never waits on
Scalar/Vector.
"""

import numpy as np
import ml_dtypes

B, S, HID = 4, 2048, 4096
H, D, L = 32, 128, 64
NCORES = 8
T = B * S
TC = T // NCORES          # tokens per core (1024)
KC = HID // 128           # 32 contraction chunks over hidden dim
HL = H * L                # folded O-proj contraction (2048)
MB = TC // 512            # 512-token m-chunks (2)
ROPE_THETA = 10000.0

S_X = 16.0                # fp8 scale on xT
S_Q = 8192.0              # fp8 scale on WqT (1/sqrt(D) already folded)
S_P = 16.0                # fp8 scale on qa/qb (rope products)
S_K = 16.0                # fp8 scale on adapter KT
S_PR = 64.0               # fp8 scale on probs (folded via 1/64 indicator)
S_VW = 1024.0             # fp8 scale on VWo
QSCALE = S_P / (S_X * S_Q)   # folded into the cos/sin tables on host
ESCALE = 1.0 / (S_P * S_K)   # descale via the exp activation's scale arg
OSCALE = 1.0 / (S_PR * S_VW)
SWI = True                   # DoubleRowSwInterleave weight layout (fast LDW)

_cache = {}


def _build(tc_tokens=TC):
    """Builds the SPMD Bass graph (identical on all 8 cores)."""
    import concourse.tile as tile
    from concourse import bacc, mybir
    from contextlib import ExitStack

    bf16 = mybir.dt.bfloat16
    fp8 = mybir.dt.float8e4
    f32 = mybir.dt.float32
    MUL = mybir.AluOpType.mult
    ADD = mybir.AluOpType.add
    EXP = mybir.ActivationFunctionType.Exp
    DR = (mybir.MatmulPerfMode.DoubleRowSwInterleave if SWI
          else mybir.MatmulPerfMode.DoubleRow)

    assert tc_tokens == TC and MB == 2

    nc = bacc.Bacc(
        "TRN2",
        target_bir_lowering=False,
        debug=False,
        enable_asserts=False,
    )

    # Host-pretiled layouts (every DMA a large contiguous burst):
    #   xT    [128, KC*tc]   : DoubleRow rhs tiling (see tile_dr_rhs)
    #   wqT   [H*128, KC*128]: [128h+p, 256k2+128i+c] = (Wq.T*s)[256k2+128i+p, 128h+c]
    #   ktpT  [128, H*2*L]   : [p, 128h+64i+l] = KT_h[p,l] (i=0) / KT_h[(p+64)%128,l] (i=1)
    #   vwoT  [KC*128, HL]   : [128n+p, 256b+128i+c] = (VWo*s)[256b+128i+p, 128n+c]
    xT = nc.dram_tensor("xT", [128, KC * TC], fp8, kind="ExternalInput").ap()
    cosT = nc.dram_tensor("cosT", [D, TC], bf16, kind="ExternalInput").ap()
    sinT = nc.dram_tensor("sinT", [D, TC], bf16, kind="ExternalInput").ap()
    wqT = nc.dram_tensor("wqT", [H * 128, KC * 128], fp8, kind="ExternalInput").ap()
    ktpT = nc.dram_tensor("ktpT", [128, H * 2 * L], fp8, kind="ExternalInput").ap()
    vwoT = nc.dram_tensor("vwoT", [KC * 128, HL], fp8, kind="ExternalInput").ap()
    baseT = nc.dram_tensor("baseT", [HID, TC], bf16, kind="ExternalInput").ap()
    u2T = nc.dram_tensor("u2T", [128, 64], bf16, kind="ExternalInput").ap()
    z2T = nc.dram_tensor("z2T", [32, 33 * 64], bf16, kind="ExternalInput").ap()
    outT = nc.dram_tensor("outT", [HID, TC], f32, kind="ExternalOutput").ap()

    with tile.TileContext(nc) as tc, ExitStack() as ctx:
        const_pool = ctx.enter_context(tc.tile_pool(name="const", bufs=1))
        persist = ctx.enter_context(tc.tile_pool(name="persist", bufs=1))

        # ---- persistent SBUF residents ----
        xT_sb = persist.tile([128, KC * TC], fp8)
        cos_sb = persist.tile([128, TC], bf16)
        sin_sb = persist.tile([128, TC], bf16)
        ktp_sb = persist.tile([128, H * 2 * L], fp8)
        probsT_sb = persist.tile([128, 16 * 1024], fp8)   # [p, 1024a+512m+t]
        vwo_sb = persist.tile([128, KC * HL], fp8)        # [p, 2048ni+col]
        U2 = const_pool.tile([128, 64], bf16)
        Z2 = const_pool.tile([32, 33 * 64], bf16)

        def late_loads():
            nc.sync.dma_start(cos_sb[:], cosT[:])
            nc.sync.dma_start(sin_sb[:], sinT[:])
            nc.sync.dma_start(ktp_sb[:], ktpT[:])
            nc.sync.dma_start(U2[:], u2T[:])
            nc.sync.dma_start(Z2[:], z2T[:])
            for k2 in range(1, KC // 2):
                cs = slice(2048 * k2, 2048 * (k2 + 1))
                nc.sync.dma_start(xT_sb[:, cs], xT[:, cs])

        xT_r = xT_sb.rearrange("p (k q i m) -> p k q i m", k=KC // 2, q=MB, i=2)
        ktp_r = ktp_sb.rearrange("p (h i l) -> p h i l", h=H, i=2)

        # ============ Phase B: Qproj + RoPE + scores + softmax ============
        with tc.tile_pool(name="wq", bufs=2) as wqp, \
             tc.tile_pool(name="rope", bufs=10) as rp, \
             tc.tile_pool(name="esb", bufs=34) as esbp, \
             tc.tile_pool(name="recs", bufs=4) as recp, \
             tc.tile_pool(name="fin", bufs=3) as fin, \
             tc.tile_pool(name="sups", bufs=1, space="PSUM") as sup, \
             tc.tile_pool(name="bcps", bufs=2, space="PSUM") as bcp:

            qab_st, esb_st, sums_st, rec_st, wq_st = {}, {}, {}, {}, {}

            def wq_load(j):
                wq_sb = wqp.tile([128, KC * 128], fp8, tag="wq")
                nc.sync.dma_start(wq_sb[:], wqT[128 * j:128 * (j + 1), :])
                wq_st[j] = wq_sb

            def qproj(a):
                for j in (2 * a, 2 * a + 1):
                    wq_sb = wq_st.pop(j)
                    wq_r = wq_sb.rearrange("p (k i c) -> p k i c",
                                           k=KC // 2, i=2)
                    for m in range(MB):
                        qp = qpsp.tile([128, 512], f32, tag="qp",
                                       name=f"qp{j}_{m}")
                        for k2 in range(KC // 2):
                            lhsT = (wq_sb[:, 256 * k2:256 * (k2 + 1)]
                                    if SWI else wq_r[:, k2])
                            nc.tensor.matmul(
                                qp[:], lhsT, xT_r[:, k2, m],
                                start=(k2 == 0), stop=(k2 == KC // 2 - 1),
                                perf_mode=DR,
                            )
                        ms = slice(512 * m, 512 * (m + 1))
                        qab = rp.tile([128, 1024], fp8, tag="qab",
                                      name=f"qab{j}_{m}")
                        nc.vector.tensor_tensor(qab[:, 0:512], qp[:],
                                                cos_sb[:, ms], MUL)
                        nc.vector.tensor_tensor(qab[:, 512:1024], qp[:],
                                                sin_sb[:, ms], MUL)
                        qab_st[(j, m)] = qab

            def scores(a):
                # DoubleRow can't col-tile (XBUS budget), so the two RoPE
                # arms accumulate as two plain fp8 matmuls per head; the two
                # heads of a pair land in col-groups 0-63 / 64-127.
                for m in range(MB):
                    psc = scp.tile([128, 512], f32, tag="sc", name=f"sc{a}_{m}")
                    for i, j in enumerate((2 * a, 2 * a + 1)):
                        qab = qab_st.pop((j, m))
                        for arm in range(2):
                            nc.tensor.matmul(
                                psc[64 * i:64 * (i + 1), :],
                                ktp_r[:, j, arm, :],
                                qab[:, 512 * arm:512 * (arm + 1)],
                                start=(arm == 0), stop=(arm == 1),
                            )
                    esb = esbp.tile([128, 512], bf16, tag="esb",
                                    name=f"esb{a}_{m}")
                    nc.scalar.activation(esb[:], psc[:], EXP, scale=ESCALE)
                    esb_st[(a, m)] = esb

            def sums(a):
                # Both m-chunks of a group share one [32,512] PSUM bank
                # (head 2al+i of chunk m at row 2al+16m+i) -> one reciprocal
                # per 16-head group.
                g, al = a // 8, a % 8
                if al == 0:
                    sums_st[g] = sup.tile([32, 512], f32, tag="sums",
                                          name=f"su{g}")
                for m in range(MB):
                    r0 = 2 * al + 16 * m
                    nc.tensor.matmul(
                        sums_st[g][:], U2[:, 31 - r0:63 - r0],
                        esb_st[(a, m)][:],
                        start=(al == 0 and m == 0), stop=(al == 7 and m == 1),
                    )

            def rec(g):
                rc = recp.tile([32, 512], bf16, tag="rec", name=f"rec{g}")
                with nc.allow_low_precision(reason="bf16 softmax weights"):
                    nc.vector.reciprocal(rc[:], sums_st[g][:])
                rec_st[g] = rc

            def bc_probs(a):
                g, al = a // 8, a % 8
                for m in range(MB):
                    r0 = 2 * al + 16 * m
                    pb = bcp.tile([128, 512], f32, tag="bcp", name=f"bc{a}_{m}")
                    nc.tensor.matmul(pb[:], Z2[:, 64 * r0:64 * r0 + 128],
                                     rec_st[g][:], start=True, stop=True)
                    ps = slice(1024 * a + 512 * m, 1024 * a + 512 * (m + 1))
                    nc.vector.tensor_tensor(probsT_sb[:, ps],
                                            esb_st.pop((a, m))[:], pb[:], MUL)

            # Qproj/scores PSUM pools nest so their 4 banks free up for the
            # O-proj tiles that overlap the group-1 softmax tail.
            with tc.tile_pool(name="qps", bufs=2, space="PSUM") as qpsp, \
                 tc.tile_pool(name="scps", bufs=2, space="PSUM") as scp:
                wq_load(0)
                nc.sync.dma_start(xT_sb[:, 0:2048], xT[:, 0:2048])
                wq_load(1)
                late_loads()
                for a in range(16):
                    qproj(a)
                    if a < 15:      # prefetch next pair's weights
                        wq_load(2 * a + 2)
                        wq_load(2 * a + 3)
                    # stream the resident O-proj weights in during phase B
                    for ni in (2 * a, 2 * a + 1):
                        nc.sync.dma_start(
                            vwo_sb[:, HL * ni:HL * (ni + 1)],
                            vwoT[128 * ni:128 * (ni + 1), :])
                    if a >= 1:
                        scores(a - 1)
                    if a == 10:
                        rec(0)  # before sums(8) so group-0 PSUM slots free
                    if a >= 2:
                        sums(a - 2)
                    if a >= 12:
                        bc_probs(2 * (a - 12))
                        bc_probs(2 * (a - 12) + 1)
                scores(15)
                sums(14)
                sums(15)

            # ============ Phase C: fp8 DoubleRow O-proj + base add ========
            # ni 0/1 accumulate their group-0 chunks (b 0-3) while the
            # group-1 reciprocal/broadcast/probs tail is still running.
            with tc.tile_pool(name="ops", bufs=4, space="PSUM") as opp:
                pr_r = probsT_sb.rearrange("p (b i m t) -> p b i m t",
                                           b=8, i=2, m=MB)
                vwo_r = vwo_sb.rearrange("p (n b i c) -> p n b i c",
                                         n=KC, b=8, i=2)

                def omm(op, ni, m, b, start, stop):
                    lhsT = (vwo_sb[:, HL * ni + 256 * b:HL * ni + 256 * (b + 1)]
                            if SWI else vwo_r[:, ni, b])
                    nc.tensor.matmul(op[:], lhsT, pr_r[:, b, :, m, :],
                                     start=start, stop=stop, perf_mode=DR)

                def ofin(ni, osb, op, m, bt):
                    ts = slice(512 * m, 512 * (m + 1))
                    nc.vector.scalar_tensor_tensor(
                        osb[:, ts], op[:], OSCALE, bt[:, ts], MUL, ADD)

                op_st, bt_st = {}, {}
                for ni in (0, 1):
                    bt = fin.tile([128, TC], bf16, tag="bt", name=f"bt{ni}")
                    nc.sync.dma_start(bt[:], baseT[128 * ni:128 * (ni + 1), :])
                    bt_st[ni] = bt
                    for m in range(MB):
                        op = opp.tile([128, 512], f32, tag="op",
                                      name=f"op{ni}_{m}")
                        op_st[(ni, m)] = op
                        for b in range(4):
                            omm(op, ni, m, b, b == 0, False)
                rec(1)
                for a in range(8, 16):
                    bc_probs(a)
                for ni in (0, 1):
                    osb = fin.tile([128, TC], f32, tag="osb")
                    bt = bt_st.pop(ni)
                    for m in range(MB):
                        op = op_st.pop((ni, m))
                        for b in range(4, 8):
                            omm(op, ni, m, b, False, b == 7)
                        ofin(ni, osb, op, m, bt)
                    nc.sync.dma_start(outT[128 * ni:128 * (ni + 1), :], osb[:])

                for ni in range(2, KC):
                    bt = fin.tile([128, TC], bf16, tag="bt")
                    nc.sync.dma_start(bt[:], baseT[128 * ni:128 * (ni + 1), :])
                    osb = fin.tile([128, TC], f32, tag="osb")
                    for m in range(MB):
                        op = opp.tile([128, 512], f32, tag="op",
                                      name=f"op{ni}_{m}")
                        for b in range(8):
                            omm(op, ni, m, b, b == 0, b == 7)
                        ofin(ni, osb, op, m, bt)
                    nc.sync.dma_start(
                        outT[128 * ni:128 * (ni + 1), :], osb[:])

    nc.compile()
    return nc


def _host_prep(hidden_states, base_output, Wq, Wk, Wv, Wo, adaption_prompt,
               adaption_gate, position_ids, tc_tokens=TC, ncores=NCORES):
    bf16 = ml_dtypes.bfloat16
    fp8 = ml_dtypes.float8_e4m3
    f32 = np.float32

    def to_fp8(a):
        return np.clip(a, -240.0, 240.0).astype(fp8)

    x = np.ascontiguousarray(np.asarray(hidden_states, f32).reshape(T, HID))
    base = np.asarray(base_output, f32).reshape(T, HID)
    pos = np.asarray(position_ids).reshape(T).astype(np.int64)

    inv = 1.0 / (ROPE_THETA ** (np.arange(0, D, 2, dtype=f32) / D))
    freqs = pos[:, None].astype(f32) * inv[None, :]          # [T, 64]
    emb = np.concatenate([freqs, freqs], axis=1)             # [T, 128]
    # QSCALE compensates the fp8 scaling of the Q projection inputs
    cos = (np.cos(emb) * QSCALE).astype(f32)
    sin = (np.sin(emb) * QSCALE).astype(f32)
    # sin arm pairs with the row-swapped KT: +sin (d<64), -sin (d>=64)
    sin_signed = sin.copy()
    sin_signed[:, D // 2:] *= -1.0

    gate = f32(np.asarray(adaption_gate).reshape(-1)[0])
    scale = f32(1.0 / np.sqrt(D))
    prompt = np.asarray(adaption_prompt, f32).reshape(L, HID)

    def tile_doublerow(A):
        # A [K, N] -> [N, K] tiled.
        # DoubleRow:       [128n+p, 256b+128i+c]    = A[256b+128i+p, 128n+c]
        # SwInterleave:    [128n+p, 256b+2(127-c)+i] = A[256b+128i+p, 128n+c]
        K, N = A.shape
        t = A.reshape(K // 256, 2, 128, N // 128, 128).transpose(3, 2, 0, 1, 4)
        if SWI:                       # (n, p, b, i, c) -> (n, p, b, 127-c, i)
            t = t[..., ::-1].transpose(0, 1, 2, 4, 3)
        return np.ascontiguousarray(t.reshape(N, K))

    def tile_dr_rhs(A):
        # A [HID, n] -> [128, KC*n], cols (k2, mc, i, m):
        # [p, k2*2n + mc*1024 + i*512 + m] = A[256k2+128i+p, 512mc+m]
        n = A.shape[1]
        return np.ascontiguousarray(
            A.reshape(KC // 2, 2, 128, n // 512, 512)
             .transpose(2, 0, 3, 1, 4).reshape(128, KC * n))

    WqT = tile_doublerow(np.asarray(Wq, f32).T * (scale * f32(S_Q)))
    WqT = to_fp8(WqT)

    # adapter K (host fold): K = prompt @ Wk.T, per head [KT | KT row-swapped]
    Kmat = prompt @ np.asarray(Wk, f32).T                    # [L, HID]
    KT = Kmat.reshape(L, H, D).transpose(2, 1, 0)            # [D, H, L]
    ktp = np.empty((128, H, 2, L), f32)
    ktp[:, :, 0, :] = KT * S_K
    ktp[:, :, 1, :] = np.roll(KT, -D // 2, axis=0) * S_K
    ktpT = to_fp8(ktp.reshape(128, H * 2 * L))

    # folded V@Wo (host): VWo[h] = (prompt @ Wv.T * gate)[:, h] @ Wo.T[h]
    V = (prompt @ np.asarray(Wv, f32).T) * gate              # [L, HID]
    V5 = V.reshape(L, H, D).transpose(1, 0, 2)               # [H, L, D]
    WoT5 = np.asarray(Wo, f32).T.reshape(H, D, HID)          # [H, D, HID]
    M2 = (V5 @ WoT5).reshape(HL, HID)                        # [(h,l), HID]
    vwoT = to_fp8(tile_doublerow(M2 * f32(S_VW)))

    u2 = np.zeros((128, 64), f32)
    u2[0:64, 31] = 1.0 / S_PR
    u2[64:128, 32] = 1.0 / S_PR
    z2 = np.zeros((32, 33 * 64), f32)
    for k in range(32):
        z2[k, 64 * k:64 * (k + 1)] = 1.0
    u2 = u2.astype(bf16)
    z2 = z2.astype(bf16)

    in_maps = []
    for c in range(ncores):
        lo = c * tc_tokens
        hi = lo + tc_tokens
        in_maps.append({
            "xT": tile_dr_rhs(to_fp8(x[lo:hi].T * f32(S_X))),
            "cosT": np.ascontiguousarray(cos[lo:hi].T).astype(bf16),
            "sinT": np.ascontiguousarray(sin_signed[lo:hi].T).astype(bf16),
            "wqT": WqT,
            "ktpT": ktpT,
            "vwoT": vwoT,
            "baseT": np.ascontiguousarray(base[lo:hi].T).astype(bf16),
            "u2T": u2,
            "z2T": z2,
        })
    return in_maps


def kernel(hidden_states, base_output, Wq, Wk, Wv, Wo, adaption_prompt,
           adaption_gate, position_ids):
    from concourse import bass_utils

    if "nc" not in _cache:
        _cache["nc"] = _build()
    nc = _cache["nc"]

    in_maps = _host_prep(hidden_states, base_output, Wq, Wk, Wv, Wo,
                         adaption_prompt, adaption_gate, position_ids)

    res = bass_utils.run_bass_kernel_spmd(nc, in_maps, core_ids=list(range(NCORES)))

    out = np.empty((T, HID), np.float32)
    for c in range(NCORES):
        out[c * TC:(c + 1) * TC] = res.results[c]["outT"].T
    return out.reshape(B, S, HID)
